# revision 17
# baseline (speedup 1.0000x reference)
# Multi-head attention layer on 8 TRN2 NeuronCores (SPMD, no collectives).
#
# Problem: B=4, N=2048, D=512, H=8 heads (DK=64).
#   out = softmax((q@Wq+bq)(k@Wk+bk)^T / 8) (v@Wv+bv) @ Wo + bo   per (batch, head)
#
# Sharding: core c handles batch b=c//2 and query-row half c%2 (1024 rows).
# K/V projections are recomputed by both cores of a pair (cheap) so there is
# no cross-core communication at all.
#
# v2 over the baseline (229us): the trace showed PE 86% busy (203us) as the
# bottleneck, ACT 62% (147us exp stream), DVE 31%.  Three structural changes:
#
# 1. ROW-TILED S PAIRS.  The S^T matmul contracts over DK=64 — half the PE
#    array.  Head-even (d on partitions 0:64) and head-odd (64:128) S matmuls
#    are emitted adjacently; bass auto-derives tile_position (0,0)/(64,0)
#    (64x128 row-tiling mode T0/T8) and the PE runs them CONCURRENTLY.
#    S time halves: ~55us -> ~27us of PE busy.
#
# 2. EXP SPLIT ACT/DVE.  Per pair-kt two [128,1024] score tiles need exp.
#    Odd-head tiles: exact ACT exp (as before).  Even-head tiles: a one-op
#    DVE "bit trick" exp — y = x*(2^7/ln2)/8 + BIG in fp32, where
#    BIG = 1.5*2^23 + (127<<7) - 8 forces fixed-point rounding so the LOW
#    half-word of each f32 y IS the bf16 bit pattern of exp(x/8) (max rel
#    err ~4.5%, mean 1.5%, washes out under softmax normalization).  The AV
#    matmul reads the bf16 values through a strided bitcast view, so one
#    1.19us DVE op replaces a 1.15us ACT op — both engines stream exps in
#    parallel and exp never gates the PE.
#
# 3. SEQUENTIAL AV CHAINS + SINGLE-COPY HANDOFF.  AV for head-even drains
#    during its own phase (lag ~3 slots), head-odd's pts buffer in SBUF and
#    drain during the next phase, both through ONE PSUM ctx accumulator.
#    At chain end one ACT copy moves [128,1024] (ctx rows + ones-replicated
#    denominator) PSUM->SBUF; the reciprocal/shift/multiply tail runs a few
#    slots later as before.
#
# PSUM: s-ring 3 x [128,1024] (6 banks, shared with projections) + 1 ctx
# (2 banks) = 8 banks exactly.
from contextlib import ExitStack

import numpy as np
import ml_dtypes

import concourse.bass as bass
import concourse.mybir as mybir
import concourse.tile as tile
from concourse import bacc
from concourse.bass_utils import run_bass_kernel_spmd

BF16 = mybir.dt.bfloat16
F32 = mybir.dt.float32
Exp = mybir.ActivationFunctionType.Exp
MULT = mybir.AluOpType.mult
ADD = mybir.AluOpType.add

B, N, D, H = 4, 2048, 512, 8
DK = D // H          # 64
NQ = N // 2          # 1024 query rows per core
NKT = N // 128       # 16 k tiles
NP = H // 2          # 4 head pairs

# exp bit-trick constants: low16(fp32(x*K8 + BIG)) == bf16 bits of exp(x/8)
K8 = float(np.float32(128.0 / np.log(2.0) / 8.0))          # 23.0827
BIG = float(np.float32(1.5 * 2**23 + (127 << 7) - 8))      # fixed-point + bias


def build_nc():
    nc = bacc.Bacc("TRN2", target_bir_lowering=False)

    qT = nc.dram_tensor("qT", (D, NQ), BF16, kind="ExternalInput")
    kT = nc.dram_tensor("kT", (D, N), BF16, kind="ExternalInput")
    vT = nc.dram_tensor("vT", (D, N), BF16, kind="ExternalInput")
    wq = nc.dram_tensor("wq", (D, D), BF16, kind="ExternalInput")
    wk = nc.dram_tensor("wk", (D, D), BF16, kind="ExternalInput")
    wv = nc.dram_tensor("wv", (D, D), BF16, kind="ExternalInput")
    wo = nc.dram_tensor("wo", (D, D), BF16, kind="ExternalInput")
    bq = nc.dram_tensor("bq", (D, 1), F32, kind="ExternalInput")
    bk = nc.dram_tensor("bk", (D, 1), F32, kind="ExternalInput")
    bv = nc.dram_tensor("bv", (1, D), BF16, kind="ExternalInput")
    bo = nc.dram_tensor("bo", (1, D), BF16, kind="ExternalInput")
    out = nc.dram_tensor("out", (NQ, D), BF16, kind="ExternalOutput")

    with tile.TileContext(nc) as tc:
        with ExitStack() as ctx:
            emit(ctx, tc, qT, kT, vT, wq, wk, wv, wo, bq, bk, bv, bo, out)
    nc.compile()
    return nc


def emit(ctx, tc, qT, kT, vT, wq, wk, wv, wo, bq, bk, bv, bo, out):
    nc = tc.nc
    consts = ctx.enter_context(tc.tile_pool(name="consts", bufs=1))
    # odd-head pts (ACT exp, bf16) live up to a full phase before their AV
    p_pool = ctx.enter_context(tc.tile_pool(name="p_pool", bufs=17))
    # even-head DVE-trick tiles (f32, bitcast-read); consumed within ~3 slots
    y_pool = ctx.enter_context(tc.tile_pool(name="y_pool", bufs=4))
    post = ctx.enter_context(tc.tile_pool(name="post", bufs=1))
    outs = ctx.enter_context(tc.tile_pool(name="outs", bufs=4))
    # PSUM: shared 3-deep s-ring (S pairs + projections, 6 banks) + 1 ctx
    # (2 banks) = 8 banks.  (A per-parity 1-deep split was tried and lost
    # ~19us: exp latency ~1.2us needs >=1.5 slots of ring slack.)
    s_pool = ctx.enter_context(tc.tile_pool(name="s_pool", bufs=3, space="PSUM"))
    c_pool = ctx.enter_context(tc.tile_pool(name="c_pool", bufs=1, space="PSUM"))
    dram = ctx.enter_context(tc.tile_pool(name="dram", bufs=1, space="DRAM"))

    # ---- inputs (DMA order = first-use order; big tensors in halves) -------
    def load(name, shape, dt_, src_ap, eng=None):
        t = consts.tile(shape, dt_, name=name)
        (eng or nc.sync).dma_start(out=t, in_=src_ap)
        return t

    def load_halves(name, shape, dt_, dram_t, n, parts=2):
        t = consts.tile(shape, dt_, name=name)
        h = n // parts
        for i in range(parts):
            nc.sync.dma_start(
                out=t[:, :, i * h:(i + 1) * h],
                in_=dram_t[:, i * h:(i + 1) * h].rearrange(
                    "(c p) n -> p c n", p=128))
        return t

    def load_part(t, dram_t, n0, n1):
        nc.sync.dma_start(
            out=t[:, :, n0:n1],
            in_=dram_t[:, n0:n1].rearrange("(c p) n -> p c n", p=128))

    wk_s = load("wk_s", [128, 4, D], BF16, wk[:].rearrange("(c p) d -> p c d", p=128))
    kT_s = consts.tile([128, 4, N], BF16, name="kT_s")
    load_part(kT_s, kT, 0, 512)
    load_part(kT_s, kT, 512, 1024)
    bk_s = load("bk_s", [128, 4, 1], F32, bk[:].rearrange("(c p) o -> p c o", p=128))
    wq_s = load("wq_s", [128, 4, D], BF16, wq[:].rearrange("(c p) d -> p c d", p=128))
    qT_s = load_halves("qT_s", [128, 4, NQ], BF16, qT, NQ)
    bq_s = load("bq_s", [128, 4, 1], F32, bq[:].rearrange("(c p) o -> p c o", p=128))
    wv_s = load("wv_s", [128, 4, D], BF16, wv[:].rearrange("(c p) d -> p c d", p=128))
    bv_bc = load("bv_bc", [128, D], BF16, bv[:].to_broadcast((128, D)))
    load_part(kT_s, kT, 1024, 1536)
    load_part(kT_s, kT, 1536, 2048)
    vT_s = load_halves("vT_s", [128, 4, N], BF16, vT, N, parts=4)
    wo_s = load("wo_s", [128, 4, D], BF16, wo[:].rearrange("(c p) d -> p c d", p=128))
    bo_s = load("bo_s", [1, D], BF16, bo[:])

    ones1 = consts.tile([1, 128], BF16)
    nc.vector.memset(ones1, 1.0)
    ztile = consts.tile([128, 512], BF16)
    nc.vector.memset(ztile, 0.0)

    # tiny dummy exp: pulls the ~2.7us ACT_TABLE_LOAD into the DMA window
    tl = consts.tile([128, 16], F32)
    nc.scalar.activation(tl, ztile[:, 0:16], Exp, scale=1.0)

    KT_s = consts.tile([128, 4, N], BF16)     # K^T, d on partitions
    QT_s = consts.tile([128, 4, NQ], BF16)    # Q^T, d on partitions
    # V with k on partitions; per (kt, head) a 128-wide stationary block:
    # even heads [V(64) | ones(64)], odd heads [ones|V].  The ones half
    # replicates the softmax denominator onto the 64 partitions opposite
    # the ctx rows, so normalization needs no partition broadcast.
    V_s = consts.tile([128, NKT, H, 128], BF16)
    ctxn_s = consts.tile([128, 4, NQ], BF16)  # normalized ctx^T

    V_pairs = V_s[:].rearrange("p t (j par) w -> p t par j w", par=2)
    nc.vector.memset(V_pairs[:, :, 0, :, 64:128], 1.0)  # even heads: ones right
    nc.vector.memset(V_pairs[:, :, 1, :, 0:64], 1.0)    # odd heads: ones left

    # ---- projections --------------------------------------------------------
    def emit_kproj_half(dt, kh):  # one kT half: 8 MMs, one st ring slot
        st = s_pool.tile([128, 1024], F32, tag="s", name="st_k")
        for kc in range(2):
            for cc in range(4):
                nc.tensor.matmul(
                    st[:, kc * 512:(kc + 1) * 512],
                    lhsT=wk_s[:, cc, dt * 128:(dt + 1) * 128],
                    rhs=kT_s[:, cc, kh * 1024 + kc * 512:
                             kh * 1024 + (kc + 1) * 512],
                    start=(cc == 0), stop=(cc == 3))
        nc.scalar.add(
            KT_s[:, dt, kh * 1024:(kh + 1) * 1024], st, bk_s[:, dt, :])

    def emit_qproj(dt):  # 8 MMs, one st ring slot
        st = s_pool.tile([128, 1024], F32, tag="s", name="st_q")
        for qc in range(2):
            for cc in range(4):
                nc.tensor.matmul(
                    st[:, qc * 512:(qc + 1) * 512],
                    lhsT=wq_s[:, cc, dt * 128:(dt + 1) * 128],
                    rhs=qT_s[:, cc, qc * 512:(qc + 1) * 512],
                    start=(cc == 0), stop=(cc == 3))
        nc.scalar.add(QT_s[:, dt, :], st, bq_s[:, dt, :])

    def emit_vproj_pair(g):  # V projection for k tiles 2g, 2g+1
        st = s_pool.tile([128, 1024], F32, tag="s", name="st_v")
        for sub in range(2):
            kt = 2 * g + sub
            sl = st[:, sub * 512:(sub + 1) * 512]
            for cc in range(4):
                nc.tensor.matmul(
                    sl,
                    lhsT=vT_s[:, cc, kt * 128:(kt + 1) * 128],
                    rhs=wv_s[:, cc, :],
                    start=(cc == 0), stop=(cc == 3))
            sl_pairs = sl.rearrange("p (j par w) -> p par j w", par=2, w=64)
            bv_pairs = bv_bc[:].rearrange("p (j par w) -> p par j w",
                                          par=2, w=64)
            vt_pairs = V_s[:, kt].rearrange("p (j par) w -> p par j w", par=2)
            nc.vector.tensor_add(
                vt_pairs[:, 0, :, 0:64], sl_pairs[:, 0], bv_pairs[:, 0])
            nc.vector.tensor_add(
                vt_pairs[:, 1, :, 64:128], sl_pairs[:, 1], bv_pairs[:, 1])

    # ---- attention ----------------------------------------------------------
    def emit_s_pair(p, kt, dve_even):
        """Row-tiled S^T pair for heads (2p, 2p+1) at k-tile kt, plus exp.

        The 4 matmuls alternate partition halves (e,qc0),(o,qc0),(e,qc1),
        (o,qc1): consecutive MMs land on disjoint 64-row PE tiles (T0/T8)
        and run concurrently.  Returns (rhs_e, rhs_o): per-head AV rhs APs.
        """
        st_e = s_pool.tile([128, 1024], F32, tag="s", name="st_e")
        st_o = s_pool.tile([128, 1024], F32, tag="s", name="st_o")
        kcol = slice(kt * 128, (kt + 1) * 128)
        for qc in range(2):
            qs = slice(qc * 512, (qc + 1) * 512)
            nc.tensor.matmul(
                st_e[:, qs], lhsT=KT_s[0:64, p, kcol], rhs=QT_s[0:64, p, qs],
                start=True, stop=True)
            nc.tensor.matmul(
                st_o[:, qs], lhsT=KT_s[64:128, p, kcol], rhs=QT_s[64:128, p, qs],
                start=True, stop=True)
        if dve_even:
            y = y_pool.tile([128, 1024], F32, tag="y", name="y_e")
            nc.vector.tensor_scalar(y, st_e, K8, BIG, op0=MULT, op1=ADD)
            rhs_e = y[:].bitcast(BF16).rearrange(
                "p (n two) -> p n two", two=2)[:, :, 0]
        else:
            pt_e = p_pool.tile([128, 1024], BF16, tag="pe", bufs=2, name="pt_e")
            nc.scalar.activation(pt_e, st_e, Exp, scale=0.125)
            rhs_e = pt_e[:]
        pt_o = p_pool.tile([128, 1024], BF16, tag="p", name="pt_o")
        nc.scalar.activation(pt_o, st_o, Exp, scale=0.125)
        return rhs_e, pt_o[:]

    def emit_av(h, kt, rhs, ctx_ps):
        for qc in range(2):
            nc.tensor.matmul(
                ctx_ps[:, qc * 512:(qc + 1) * 512],
                lhsT=V_s[:, kt, h, :],
                rhs=rhs[:, qc * 512:(qc + 1) * 512],
                start=(kt == 0), stop=(kt == NKT - 1))

    # ---- per-head normalize -------------------------------------------------
    # At chain end one ACT copy stages [128,1024] (ctx half + denominator
    # half) PSUM->SBUF, freeing the single ctx accumulator.  The deferred
    # tail does reciprocal (partitions 0-63 only; partition-shift via
    # SBUF->SBUF DMA on whichever side needs it) and the multiply into ctxn.
    norm7 = {}

    def emit_handoff(h, ctx_ps):
        cc = post.tile([128, NQ], F32, tag="cc", bufs=2, name="cc")
        nc.scalar.copy(out=cc, in_=ctx_ps)
        even = (h % 2 == 0)
        cl, dl = (0, 64) if even else (64, 0)   # ctx / denom partition bases
        if h == H - 1:
            # last head: fold normalize into the output projection — stage
            # raw ctx bf16 (matmul lhsT) + reciprocals transposed onto
            # partitions via a 4KB DRAM bounce.
            def tail():
                rc = post.tile([128, NQ], F32, tag="rc", name="rc")
                nc.vector.reciprocal_approx_fast(out=rc[0:64, :],
                                                 in_=cc[0:64, :])
                ctxc = post.tile([128, NQ], BF16, tag="ctxc7", name="ctxc7")
                nc.vector.tensor_copy(out=ctxc[64:128, :], in_=cc[64:128, :])
                norm7["ctxc"] = ctxc
                dr = dram.tile([1, NQ], F32, tag="dr", name="dr")
                nc.sync.dma_start(out=dr, in_=rc[0:1, :])
                rcol = post.tile([128, 8], F32, tag="rcol", name="rcol")
                nc.sync.dma_start(
                    out=rcol,
                    in_=dr[:].rearrange("o (f p) -> (o p) f", p=128))
                norm7["rcol"] = rcol
            return tail

        def tail():
            dt = h // 2
            d = cc
            if dl != 0:
                den2 = post.tile([128, NQ], F32, tag="shift", name="den2")
                nc.sync.dma_start(out=den2[0:64, :], in_=cc[dl:dl + 64, :])
                d = den2
            rc = post.tile([128, NQ], F32, tag="rc", name="rc")
            nc.vector.reciprocal_approx_fast(out=rc[0:64, :], in_=d[0:64, :])
            if cl != 0:
                rc2 = post.tile([128, NQ], F32, tag="shift", name="rc2")
                nc.sync.dma_start(out=rc2[cl:cl + 64, :], in_=rc[0:64, :])
                rc = rc2
            nc.vector.tensor_mul(ctxn_s[cl:cl + 64, dt, :],
                                 cc[cl:cl + 64, :], rc[cl:cl + 64, :])
        return tail

    # ---- output projection (unchanged from baseline) ------------------------
    def emit_outproj_main(g, pool, tag):
        st = pool.tile([128, 1024], F32, tag=tag, name="st_o")
        for sub in range(2):
            nt = g * 2 + sub
            sl = st[:, sub * 512:(sub + 1) * 512]
            for dc in range(3):
                nc.tensor.matmul(
                    sl,
                    lhsT=ctxn_s[:, dc, nt * 128:(nt + 1) * 128],
                    rhs=wo_s[:, dc, :],
                    start=(dc == 0), stop=False)
            nc.tensor.matmul(
                sl,
                lhsT=ctxn_s[0:64, 3, nt * 128:(nt + 1) * 128],
                rhs=wo_s[0:64, 3, :],
                start=False, stop=False)
            nc.tensor.matmul(sl, lhsT=ones1, rhs=bo_s, start=False, stop=True)
        return st

    def emit_fin_pre(g, st):
        # stage the main sums to SBUF promptly (ACT; frees nothing but gets
        # the copies off the post-fold critical path)
        ots = []
        for sub in range(2):
            sl = st[:, sub * 512:(sub + 1) * 512]
            ot = outs.tile([128, D], BF16, tag="o", bufs=6, name="ot")
            nc.scalar.copy(out=ot, in_=sl)
            ots.append(ot)
        return ots

    def emit_fin_post(g, st, ots):
        for sub in range(2):
            nt = g * 2 + sub
            sl = st[:, sub * 512:(sub + 1) * 512]
            nc.tensor.matmul(
                sl,
                lhsT=norm7["ctxc"][64:128, nt * 128:(nt + 1) * 128],
                rhs=wo_s[64:128, 3, :],
                start=True, stop=True)
            ot2 = outs.tile([128, D], BF16, tag="o2", bufs=6, name="ot2")
            nc.vector.scalar_tensor_tensor(
                out=ot2, in0=sl, scalar=norm7["rcol"][:, nt:nt + 1],
                in1=ots[sub],
                op0=mybir.AluOpType.mult, op1=mybir.AluOpType.add)
            nc.sync.dma_start(out=out[nt * 128:(nt + 1) * 128, :], in_=ot2)

    # ---- schedule -----------------------------------------------------------
    # warm-up: dummy matmuls during the initial DMA window keep the PE's HAM
    # clock gate at 8/8 until the first projection matmuls are DMA-ready
    wst = s_pool.tile([128, 1024], F32, tag="s", name="wst")
    for i in range(40):
        nc.tensor.matmul(wst[:, 0:512], lhsT=ztile[:, 0:128], rhs=ztile,
                         start=(i == 0), stop=(i == 39))
    emit_kproj_half(0, 0)
    emit_qproj(0)

    # Projection prefetch placement per phase (16 pair-kt slots each).
    # DMA arrival order gates phase-0 placements (kT half 2, all of vT
    # arrive while phase 0 runs).
    def phase_mid(p):
        # K^T/Q^T for phase p+1 must complete during phase p.
        if p == 0:
            mid = {1: lambda: emit_kproj_half(0, 1),
                   10: lambda: emit_kproj_half(1, 0),
                   12: lambda: emit_kproj_half(1, 1),
                   14: lambda: emit_qproj(1)}
            for g in range(7):
                mid[3 + g] = (lambda g=g: emit_vproj_pair(g))
            mid[11] = lambda: emit_vproj_pair(7)
            return mid
        if p == 1:
            return {2: lambda: emit_kproj_half(2, 0),
                    6: lambda: emit_kproj_half(2, 1),
                    10: lambda: emit_qproj(2)}
        if p == 2:
            return {2: lambda: emit_kproj_half(3, 0),
                    6: lambda: emit_kproj_half(3, 1),
                    10: lambda: emit_qproj(3)}
        return {}

    # AV consumption: strictly chain-sequential (head 0 fully, then 1, ...)
    # through the single ctx accumulator.  prod[h][kt] = (rhs, produced_slot);
    # an entry is poppable LAG slots after production (so the PE never
    # head-blocks waiting on its exp).  Hand-off at kt==15, tail deferred to
    # the next chain's kt==4.
    LAG = 2
    prod = {h: {} for h in range(H)}
    cons = {"h": 0, "kt": 0, "n": 0, "ctx": None}
    tails = {}
    n_prod = [0]

    def next_entry():
        if cons["h"] >= H:
            return None
        return prod[cons["h"]].get(cons["kt"])

    def pop_av(now_slot):
        e = next_entry()
        if e is None or (now_slot is not None and now_slot < e[1] + LAG):
            return False
        h, kt = cons["h"], cons["kt"]
        if kt == 0:
            # last chain: take a (by-then free) s-ring slot so its first AV
            # doesn't serialize behind the previous chain's hand-off copy
            pool, tag = (s_pool, "s") if h == H - 1 else (c_pool, "c")
            cons["ctx"] = pool.tile([128, 1024], F32, tag=tag, name="ctx_ps")
        emit_av(h, kt, e[0], cons["ctx"])
        cons["n"] += 1
        if kt == 4 and h >= 1:
            t = tails.pop(h - 1, None)
            if t:
                t()
        if kt == NKT - 1:
            tails[h] = emit_handoff(h, cons["ctx"])
            cons["h"] += 1
            cons["kt"] = 0
        else:
            cons["kt"] += 1
        return True

    for p in range(NP):
        mid = phase_mid(p)
        for kt in range(NKT):
            slot = p * NKT + kt
            if kt in mid:
                mid[kt]()
            # DVE trick skipped for the first two slots only (y-ring backlog
            # is V-gated in early phase 0; ACT has slack there anyway).
            dve_even = not (p == 0 and kt < 2)
            rhs_e, rhs_o = emit_s_pair(p, kt, dve_even)
            prod[2 * p][kt] = (rhs_e, slot)
            prod[2 * p + 1][kt] = (rhs_o, slot)
            n_prod[0] += 2
            target = max(6, 14 - 2 * kt)
            while n_prod[0] - cons["n"] > target and pop_av(slot):
                pass
    # Final drain: chain 6 remainder + chain 7, with outproj mains g0-g2
    # interleaved into the AV stream (tail(6) fires at chain-7 kt4,
    # unblocking the h6 half-column they read).  g2 rides the c-ring slot
    # freed by chain 6's hand-off; g3's s-slot frees at chain 7's hand-off.
    main_cfg = [("s", s_pool), ("s", s_pool), ("c", c_pool), ("s", s_pool)]
    sts, fots = [], []

    def emit_main(g):
        tag, pool = main_cfg[g]
        st = emit_outproj_main(g, pool, tag)
        sts.append(st)
        fots.append(emit_fin_pre(g, st))

    while pop_av(None):
        if (cons["h"] == 7 and len(sts) < 3
                and cons["kt"] >= 6 + 3 * len(sts)):
            emit_main(len(sts))
    t = tails.pop(6, None)
    if t:
        t()
    tails.pop(7)()
    while len(sts) < 4:
        emit_main(len(sts))
    for g in range(4):
        emit_fin_post(g, sts[g], fots[g])


_NC_CACHE = None


def _get_nc():
    global _NC_CACHE
    if _NC_CACHE is None:
        _NC_CACHE = build_nc()
    return _NC_CACHE


def make_in_maps(query, key, value, Wq, bq, Wk, bk, Wv, bv, Wo, bo):
    bf = ml_dtypes.bfloat16
    f = np.float32
    query = np.asarray(query, f)
    key = np.asarray(key, f)
    value = np.asarray(value, f)
    shared = {
        "wq": np.asarray(Wq, f).astype(bf),
        "wk": np.asarray(Wk, f).astype(bf),
        "wv": np.asarray(Wv, f).astype(bf),
        "wo": np.asarray(Wo, f).astype(bf),
        "bq": np.asarray(bq, f).reshape(D, 1),
        "bk": np.asarray(bk, f).reshape(D, 1),
        "bv": np.asarray(bv, f).astype(bf).reshape(1, D),
        "bo": np.asarray(bo, f).astype(bf).reshape(1, D),
    }
    kTs = [np.ascontiguousarray(key[b].T).astype(bf) for b in range(B)]
    vTs = [np.ascontiguousarray(value[b].T).astype(bf) for b in range(B)]
    in_maps = []
    for c in range(8):
        b, half = c // 2, c % 2
        m = dict(shared)
        m["qT"] = np.ascontiguousarray(
            query[b, half * NQ:(half + 1) * NQ, :].T).astype(bf)
        m["kT"] = kTs[b]
        m["vT"] = vTs[b]
        in_maps.append(m)
    return in_maps


def run(inputs, trace=False):
    nc = _get_nc()
    in_maps = make_in_maps(**inputs)
    res = run_bass_kernel_spmd(nc, in_maps, core_ids=list(range(8)), trace=trace)
    out = np.empty((B, N, D), np.float32)
    for c in range(8):
        b, half = c // 2, c % 2
        out[b, half * NQ:(half + 1) * NQ, :] = np.asarray(
            res.results[c]["out"], dtype=np.float32)
    return out, res


def kernel(**inputs):
    out, _ = run(inputs, trace=False)
    return out


# revision 21
# speedup vs baseline: 1.0082x; 1.0082x over previous
# Multi-head attention layer on 8 TRN2 NeuronCores (SPMD, no collectives).
#
# Problem: B=4, N=2048, D=512, H=8 heads (DK=64).
#   out = softmax((q@Wq+bq)(k@Wk+bk)^T / 8) (v@Wv+bv) @ Wo + bo   per (batch, head)
#
# Sharding: core c handles batch b=c//2 and query-row half c%2 (1024 rows).
# K/V projections are recomputed by both cores of a pair (cheap) so there is
# no cross-core communication at all.
#
# v2 over the baseline (229us): the trace showed PE 86% busy (203us) as the
# bottleneck, ACT 62% (147us exp stream), DVE 31%.  Three structural changes:
#
# 1. ROW-TILED S PAIRS.  The S^T matmul contracts over DK=64 — half the PE
#    array.  Head-even (d on partitions 0:64) and head-odd (64:128) S matmuls
#    are emitted adjacently; bass auto-derives tile_position (0,0)/(64,0)
#    (64x128 row-tiling mode T0/T8) and the PE runs them CONCURRENTLY.
#    S time halves: ~55us -> ~27us of PE busy.
#
# 2. EXP SPLIT ACT/DVE.  Per pair-kt two [128,1024] score tiles need exp.
#    Odd-head tiles: exact ACT exp (as before).  Even-head tiles: a one-op
#    DVE "bit trick" exp — y = x*(2^7/ln2)/8 + BIG in fp32, where
#    BIG = 1.5*2^23 + (127<<7) - 8 forces fixed-point rounding so the LOW
#    half-word of each f32 y IS the bf16 bit pattern of exp(x/8) (max rel
#    err ~4.5%, mean 1.5%, washes out under softmax normalization).  The AV
#    matmul reads the bf16 values through a strided bitcast view, so one
#    1.19us DVE op replaces a 1.15us ACT op — both engines stream exps in
#    parallel and exp never gates the PE.
#
# 3. SEQUENTIAL AV CHAINS + SINGLE-COPY HANDOFF.  AV for head-even drains
#    during its own phase (lag ~3 slots), head-odd's pts buffer in SBUF and
#    drain during the next phase, both through ONE PSUM ctx accumulator.
#    At chain end one ACT copy moves [128,1024] (ctx rows + ones-replicated
#    denominator) PSUM->SBUF; the reciprocal/shift/multiply tail runs a few
#    slots later as before.
#
# PSUM: s-ring 3 x [128,1024] (6 banks, shared with projections) + 1 ctx
# (2 banks) = 8 banks exactly.
from contextlib import ExitStack

import numpy as np
import ml_dtypes

import concourse.bass as bass
import concourse.mybir as mybir
import concourse.tile as tile
from concourse import bacc
from concourse.bass_utils import run_bass_kernel_spmd

BF16 = mybir.dt.bfloat16
F32 = mybir.dt.float32
Exp = mybir.ActivationFunctionType.Exp
MULT = mybir.AluOpType.mult
ADD = mybir.AluOpType.add

B, N, D, H = 4, 2048, 512, 8
DK = D // H          # 64
NQ = N // 2          # 1024 query rows per core
NKT = N // 128       # 16 k tiles
NP = H // 2          # 4 head pairs

# exp bit-trick constants: low16(fp32(x*K8 + BIG)) == bf16 bits of exp(x/8)
K8 = float(np.float32(128.0 / np.log(2.0) / 8.0))          # 23.0827
BIG = float(np.float32(1.5 * 2**23 + (127 << 7) - 8))      # fixed-point + bias


def build_nc():
    nc = bacc.Bacc("TRN2", target_bir_lowering=False)

    qT = nc.dram_tensor("qT", (D, NQ), BF16, kind="ExternalInput")
    kT = nc.dram_tensor("kT", (D, N), BF16, kind="ExternalInput")
    vT = nc.dram_tensor("vT", (D, N), BF16, kind="ExternalInput")
    wq = nc.dram_tensor("wq", (D, D), BF16, kind="ExternalInput")
    wk = nc.dram_tensor("wk", (D, D), BF16, kind="ExternalInput")
    wv = nc.dram_tensor("wv", (D, D), BF16, kind="ExternalInput")
    wo = nc.dram_tensor("wo", (D, D), BF16, kind="ExternalInput")
    bq = nc.dram_tensor("bq", (D, 1), F32, kind="ExternalInput")
    bk = nc.dram_tensor("bk", (D, 1), F32, kind="ExternalInput")
    bv = nc.dram_tensor("bv", (1, D), BF16, kind="ExternalInput")
    bo = nc.dram_tensor("bo", (1, D), BF16, kind="ExternalInput")
    out = nc.dram_tensor("out", (NQ, D), BF16, kind="ExternalOutput")

    with tile.TileContext(nc) as tc:
        with ExitStack() as ctx:
            emit(ctx, tc, qT, kT, vT, wq, wk, wv, wo, bq, bk, bv, bo, out)
    nc.compile()
    return nc


def emit(ctx, tc, qT, kT, vT, wq, wk, wv, wo, bq, bk, bv, bo, out):
    nc = tc.nc
    consts = ctx.enter_context(tc.tile_pool(name="consts", bufs=1))
    # odd-head pts (ACT exp, bf16) live up to a full phase before their AV
    p_pool = ctx.enter_context(tc.tile_pool(name="p_pool", bufs=17))
    # even-head DVE-trick tiles (f32, bitcast-read); consumed within ~3 slots
    y_pool = ctx.enter_context(tc.tile_pool(name="y_pool", bufs=4))
    post = ctx.enter_context(tc.tile_pool(name="post", bufs=1))
    outs = ctx.enter_context(tc.tile_pool(name="outs", bufs=4))
    # PSUM: shared 3-deep s-ring (S pairs + projections, 6 banks) + 1 ctx
    # (2 banks) = 8 banks.  (A per-parity 1-deep split was tried and lost
    # ~19us: exp latency ~1.2us needs >=1.5 slots of ring slack.)
    s_pool = ctx.enter_context(tc.tile_pool(name="s_pool", bufs=3, space="PSUM"))
    c_pool = ctx.enter_context(tc.tile_pool(name="c_pool", bufs=1, space="PSUM"))
    dram = ctx.enter_context(tc.tile_pool(name="dram", bufs=1, space="DRAM"))

    # ---- inputs (DMA order = first-use order; big tensors in halves) -------
    def load(name, shape, dt_, src_ap, eng=None):
        t = consts.tile(shape, dt_, name=name)
        (eng or nc.sync).dma_start(out=t, in_=src_ap)
        return t

    def load_halves(name, shape, dt_, dram_t, n, parts=2):
        t = consts.tile(shape, dt_, name=name)
        h = n // parts
        for i in range(parts):
            nc.sync.dma_start(
                out=t[:, :, i * h:(i + 1) * h],
                in_=dram_t[:, i * h:(i + 1) * h].rearrange(
                    "(c p) n -> p c n", p=128))
        return t

    def load_part(t, dram_t, n0, n1):
        nc.sync.dma_start(
            out=t[:, :, n0:n1],
            in_=dram_t[:, n0:n1].rearrange("(c p) n -> p c n", p=128))

    wk_s = load("wk_s", [128, 4, D], BF16, wk[:].rearrange("(c p) d -> p c d", p=128))
    kT_s = consts.tile([128, 4, N], BF16, name="kT_s")
    load_part(kT_s, kT, 0, 512)
    load_part(kT_s, kT, 512, 1024)
    bk_s = load("bk_s", [128, 4, 1], F32, bk[:].rearrange("(c p) o -> p c o", p=128))
    wq_s = load("wq_s", [128, 4, D], BF16, wq[:].rearrange("(c p) d -> p c d", p=128))
    qT_s = load_halves("qT_s", [128, 4, NQ], BF16, qT, NQ)
    bq_s = load("bq_s", [128, 4, 1], F32, bq[:].rearrange("(c p) o -> p c o", p=128))
    wv_s = load("wv_s", [128, 4, D], BF16, wv[:].rearrange("(c p) d -> p c d", p=128))
    bv_bc = load("bv_bc", [128, D], BF16, bv[:].to_broadcast((128, D)))
    load_part(kT_s, kT, 1024, 1536)
    load_part(kT_s, kT, 1536, 2048)
    vT_s = load_halves("vT_s", [128, 4, N], BF16, vT, N, parts=4)
    wo_s = load("wo_s", [128, 4, D], BF16, wo[:].rearrange("(c p) d -> p c d", p=128))
    bo_s = load("bo_s", [1, D], BF16, bo[:])

    ones1 = consts.tile([1, 128], BF16)
    nc.vector.memset(ones1, 1.0)
    ztile = consts.tile([128, 512], BF16)
    nc.vector.memset(ztile, 0.0)

    # tiny dummy exp: pulls the ~2.7us ACT_TABLE_LOAD into the DMA window
    tl = consts.tile([128, 16], F32)
    nc.scalar.activation(tl, ztile[:, 0:16], Exp, scale=1.0)

    KT_s = consts.tile([128, 4, N], BF16)     # K^T, d on partitions
    QT_s = consts.tile([128, 4, NQ], BF16)    # Q^T, d on partitions
    # V with k on partitions; per (kt, head) a 128-wide stationary block:
    # even heads [V(64) | ones(64)], odd heads [ones|V].  The ones half
    # replicates the softmax denominator onto the 64 partitions opposite
    # the ctx rows, so normalization needs no partition broadcast.
    V_s = consts.tile([128, NKT, H, 128], BF16)
    ctxn_s = consts.tile([128, 4, NQ], BF16)  # normalized ctx^T

    V_pairs = V_s[:].rearrange("p t (j par) w -> p t par j w", par=2)
    nc.vector.memset(V_pairs[:, :, 0, :, 64:128], 1.0)  # even heads: ones right
    nc.vector.memset(V_pairs[:, :, 1, :, 0:64], 1.0)    # odd heads: ones left

    # ---- projections --------------------------------------------------------
    def emit_kproj_half(dt, kh):  # one kT half: 8 MMs, one st ring slot
        st = s_pool.tile([128, 1024], F32, tag="s", name="st_k")
        for kc in range(2):
            for cc in range(4):
                nc.tensor.matmul(
                    st[:, kc * 512:(kc + 1) * 512],
                    lhsT=wk_s[:, cc, dt * 128:(dt + 1) * 128],
                    rhs=kT_s[:, cc, kh * 1024 + kc * 512:
                             kh * 1024 + (kc + 1) * 512],
                    start=(cc == 0), stop=(cc == 3))
        nc.scalar.add(
            KT_s[:, dt, kh * 1024:(kh + 1) * 1024], st, bk_s[:, dt, :])

    def emit_qproj(dt):  # 8 MMs, one st ring slot
        st = s_pool.tile([128, 1024], F32, tag="s", name="st_q")
        for qc in range(2):
            for cc in range(4):
                nc.tensor.matmul(
                    st[:, qc * 512:(qc + 1) * 512],
                    lhsT=wq_s[:, cc, dt * 128:(dt + 1) * 128],
                    rhs=qT_s[:, cc, qc * 512:(qc + 1) * 512],
                    start=(cc == 0), stop=(cc == 3))
        nc.scalar.add(QT_s[:, dt, :], st, bq_s[:, dt, :])

    def emit_vproj_pair(g):  # V projection for k tiles 2g, 2g+1
        st = s_pool.tile([128, 1024], F32, tag="s", name="st_v")
        for sub in range(2):
            kt = 2 * g + sub
            sl = st[:, sub * 512:(sub + 1) * 512]
            for cc in range(4):
                nc.tensor.matmul(
                    sl,
                    lhsT=vT_s[:, cc, kt * 128:(kt + 1) * 128],
                    rhs=wv_s[:, cc, :],
                    start=(cc == 0), stop=(cc == 3))
            sl_pairs = sl.rearrange("p (j par w) -> p par j w", par=2, w=64)
            bv_pairs = bv_bc[:].rearrange("p (j par w) -> p par j w",
                                          par=2, w=64)
            vt_pairs = V_s[:, kt].rearrange("p (j par) w -> p par j w", par=2)
            nc.vector.tensor_add(
                vt_pairs[:, 0, :, 0:64], sl_pairs[:, 0], bv_pairs[:, 0])
            nc.vector.tensor_add(
                vt_pairs[:, 1, :, 64:128], sl_pairs[:, 1], bv_pairs[:, 1])

    # ---- attention ----------------------------------------------------------
    def emit_s_pair(p, kt, dve_even):
        """Row-tiled S^T pair for heads (2p, 2p+1) at k-tile kt, plus exp.

        The 4 matmuls alternate partition halves (e,qc0),(o,qc0),(e,qc1),
        (o,qc1): consecutive MMs land on disjoint 64-row PE tiles (T0/T8)
        and run concurrently.  Returns (rhs_e, rhs_o): per-head AV rhs APs.
        """
        st_e = s_pool.tile([128, 1024], F32, tag="s", name="st_e")
        st_o = s_pool.tile([128, 1024], F32, tag="s", name="st_o")
        kcol = slice(kt * 128, (kt + 1) * 128)
        for qc in range(2):
            qs = slice(qc * 512, (qc + 1) * 512)
            nc.tensor.matmul(
                st_e[:, qs], lhsT=KT_s[0:64, p, kcol], rhs=QT_s[0:64, p, qs],
                start=True, stop=True)
            nc.tensor.matmul(
                st_o[:, qs], lhsT=KT_s[64:128, p, kcol], rhs=QT_s[64:128, p, qs],
                start=True, stop=True)
        if dve_even:
            y = y_pool.tile([128, 1024], F32, tag="y", name="y_e")
            nc.vector.tensor_scalar(y, st_e, K8, BIG, op0=MULT, op1=ADD)
            rhs_e = y[:].bitcast(BF16).rearrange(
                "p (n two) -> p n two", two=2)[:, :, 0]
        else:
            pt_e = p_pool.tile([128, 1024], BF16, tag="pe", bufs=2, name="pt_e")
            nc.scalar.activation(pt_e, st_e, Exp, scale=0.125)
            rhs_e = pt_e[:]
        pt_o = p_pool.tile([128, 1024], BF16, tag="p", name="pt_o")
        nc.scalar.activation(pt_o, st_o, Exp, scale=0.125)
        return rhs_e, pt_o[:]

    def emit_av(h, kt, rhs, ctx_ps):
        for qc in range(2):
            nc.tensor.matmul(
                ctx_ps[:, qc * 512:(qc + 1) * 512],
                lhsT=V_s[:, kt, h, :],
                rhs=rhs[:, qc * 512:(qc + 1) * 512],
                start=(kt == 0), stop=(kt == NKT - 1))

    # ---- per-head normalize -------------------------------------------------
    # At chain end one ACT copy stages [128,1024] (ctx half + denominator
    # half) PSUM->SBUF, freeing the single ctx accumulator.  The deferred
    # tail does reciprocal (partitions 0-63 only; partition-shift via
    # SBUF->SBUF DMA on whichever side needs it) and the multiply into ctxn.
    norm7 = {}

    def emit_handoff(h, ctx_ps):
        cc = post.tile([128, NQ], F32, tag="cc", bufs=2, name="cc")
        nc.scalar.copy(out=cc, in_=ctx_ps)
        even = (h % 2 == 0)
        cl, dl = (0, 64) if even else (64, 0)   # ctx / denom partition bases
        if h == H - 1:
            # last head: fold normalize into the output projection — stage
            # raw ctx bf16 (matmul lhsT) + reciprocals transposed onto
            # partitions via a 4KB DRAM bounce.
            def tail():
                rc = post.tile([128, NQ], F32, tag="rc", name="rc")
                nc.vector.reciprocal_approx_fast(out=rc[0:64, :],
                                                 in_=cc[0:64, :])
                ctxc = post.tile([128, NQ], BF16, tag="ctxc7", name="ctxc7")
                nc.vector.tensor_copy(out=ctxc[64:128, :], in_=cc[64:128, :])
                norm7["ctxc"] = ctxc
                dr = dram.tile([1, NQ], F32, tag="dr", name="dr")
                nc.sync.dma_start(out=dr, in_=rc[0:1, :])
                rcol = post.tile([128, 8], F32, tag="rcol", name="rcol")
                nc.sync.dma_start(
                    out=rcol,
                    in_=dr[:].rearrange("o (f p) -> (o p) f", p=128))
                norm7["rcol"] = rcol
            return tail

        def tail():
            dt = h // 2
            d = cc
            if dl != 0:
                den2 = post.tile([128, NQ], F32, tag="shift", name="den2")
                nc.sync.dma_start(out=den2[0:64, :], in_=cc[dl:dl + 64, :])
                d = den2
            rc = post.tile([128, NQ], F32, tag="rc", name="rc")
            nc.vector.reciprocal_approx_fast(out=rc[0:64, :], in_=d[0:64, :])
            if cl != 0:
                rc2 = post.tile([128, NQ], F32, tag="shift", name="rc2")
                nc.sync.dma_start(out=rc2[cl:cl + 64, :], in_=rc[0:64, :])
                rc = rc2
            nc.vector.tensor_mul(ctxn_s[cl:cl + 64, dt, :],
                                 cc[cl:cl + 64, :], rc[cl:cl + 64, :])
        return tail

    # ---- output projection (unchanged from baseline) ------------------------
    def emit_outproj_main(g, pool, tag):
        st = pool.tile([128, 1024], F32, tag=tag, name="st_o")
        for sub in range(2):
            nt = g * 2 + sub
            sl = st[:, sub * 512:(sub + 1) * 512]
            for dc in range(3):
                nc.tensor.matmul(
                    sl,
                    lhsT=ctxn_s[:, dc, nt * 128:(nt + 1) * 128],
                    rhs=wo_s[:, dc, :],
                    start=(dc == 0), stop=False)
            nc.tensor.matmul(
                sl,
                lhsT=ctxn_s[0:64, 3, nt * 128:(nt + 1) * 128],
                rhs=wo_s[0:64, 3, :],
                start=False, stop=False)
            nc.tensor.matmul(sl, lhsT=ones1, rhs=bo_s, start=False, stop=True)
        return st

    def emit_fin_pre(g, st):
        # stage the main sums to SBUF promptly (ACT; frees nothing but gets
        # the copies off the post-fold critical path)
        ots = []
        for sub in range(2):
            sl = st[:, sub * 512:(sub + 1) * 512]
            ot = outs.tile([128, D], BF16, tag="o", bufs=8, name="ot")
            nc.scalar.copy(out=ot, in_=sl)
            ots.append(ot)
        return ots

    def emit_fin_mm(g, st, sub):
        nt = g * 2 + sub
        nc.tensor.matmul(
            st[:, sub * 512:(sub + 1) * 512],
            lhsT=norm7["ctxc"][64:128, nt * 128:(nt + 1) * 128],
            rhs=wo_s[64:128, 3, :],
            start=True, stop=True)

    def emit_fin_merge(g, st, ots, sub):
        # merge = (h7_partial * recip7[n]) + main: the scale rides the ACT
        # copy (per-partition scale AP), the add is a cheap bf16 DVE op —
        # splitting engines keeps the 8 merges off a single serial DVE queue
        nt = g * 2 + sub
        sl = st[:, sub * 512:(sub + 1) * 512]
        tm = outs.tile([128, D], BF16, tag="tm", bufs=2, name="tm")
        nc.scalar.activation(tm, sl, mybir.ActivationFunctionType.Copy,
                             scale=norm7["rcol"][:, nt:nt + 1])
        ot2 = outs.tile([128, D], BF16, tag="o2", bufs=2, name="ot2")
        nc.vector.tensor_add(ot2, tm, ots[sub])
        nc.sync.dma_start(out=out[nt * 128:(nt + 1) * 128, :], in_=ot2)

    # ---- schedule -----------------------------------------------------------
    # warm-up: dummy matmuls during the initial DMA window keep the PE's HAM
    # clock gate at 8/8 until the first projection matmuls are DMA-ready
    wst = s_pool.tile([128, 1024], F32, tag="s", name="wst")
    for i in range(40):
        nc.tensor.matmul(wst[:, 0:512], lhsT=ztile[:, 0:128], rhs=ztile,
                         start=(i == 0), stop=(i == 39))
    emit_kproj_half(0, 0)
    emit_qproj(0)

    # Projection prefetch placement per phase (16 pair-kt slots each).
    # DMA arrival order gates phase-0 placements (kT half 2, all of vT
    # arrive while phase 0 runs).
    def phase_mid(p):
        # K^T/Q^T for phase p+1 must complete during phase p.
        # kproj(p, 1) covers k-tiles 8-15 of phase p, so it can run INSIDE
        # phase p (by slot ~7) — sheds load from the crowded previous phase.
        if p == 0:
            mid = {1: lambda: emit_kproj_half(0, 1),
                   12: lambda: emit_kproj_half(1, 0),
                   14: lambda: emit_qproj(1)}
            for g in range(7):
                mid[3 + g] = (lambda g=g: emit_vproj_pair(g))
            mid[11] = lambda: emit_vproj_pair(7)
            return mid
        if p == 1:
            return {2: lambda: emit_kproj_half(1, 1),
                    6: lambda: emit_kproj_half(2, 0),
                    10: lambda: emit_qproj(2)}
        if p == 2:
            return {2: lambda: emit_kproj_half(2, 1),
                    6: lambda: emit_kproj_half(3, 0),
                    10: lambda: emit_qproj(3)}
        return {3: lambda: emit_kproj_half(3, 1)}

    # AV consumption: strictly chain-sequential (head 0 fully, then 1, ...)
    # through the single ctx accumulator.  prod[h][kt] = (rhs, produced_slot);
    # an entry is poppable LAG slots after production (so the PE never
    # head-blocks waiting on its exp).  Hand-off at kt==15, tail deferred to
    # the next chain's kt==4.
    LAG = 2
    prod = {h: {} for h in range(H)}
    cons = {"h": 0, "kt": 0, "n": 0, "ctx": None}
    tails = {}
    n_prod = [0]

    def next_entry():
        if cons["h"] >= H:
            return None
        return prod[cons["h"]].get(cons["kt"])

    def pop_av(now_slot):
        e = next_entry()
        if e is None or (now_slot is not None and now_slot < e[1] + LAG):
            return False
        h, kt = cons["h"], cons["kt"]
        if kt == 0:
            # last chain: take a (by-then free) s-ring slot so its first AV
            # doesn't serialize behind the previous chain's hand-off copy
            pool, tag = (s_pool, "s") if h == H - 1 else (c_pool, "c")
            cons["ctx"] = pool.tile([128, 1024], F32, tag=tag, name="ctx_ps")
        emit_av(h, kt, e[0], cons["ctx"])
        cons["n"] += 1
        if kt == 4 and h >= 1:
            t = tails.pop(h - 1, None)
            if t:
                t()
        if kt == NKT - 1:
            tails[h] = emit_handoff(h, cons["ctx"])
            cons["h"] += 1
            cons["kt"] = 0
        else:
            cons["kt"] += 1
        return True

    for p in range(NP):
        mid = phase_mid(p)
        for kt in range(NKT):
            slot = p * NKT + kt
            if kt in mid:
                mid[kt]()
            # DVE trick skipped for the first two slots only (y-ring backlog
            # is V-gated in early phase 0; ACT has slack there anyway).
            dve_even = not (p == 0 and kt < 2)
            rhs_e, rhs_o = emit_s_pair(p, kt, dve_even)
            prod[2 * p][kt] = (rhs_e, slot)
            prod[2 * p + 1][kt] = (rhs_o, slot)
            n_prod[0] += 2
            target = max(6, 14 - 2 * kt)
            while n_prod[0] - cons["n"] > target and pop_av(slot):
                pass
    # Final drain: chain 6 remainder + chain 7, with outproj mains g0-g2
    # interleaved into the AV stream (tail(6) fires at chain-7 kt4,
    # unblocking the h6 half-column they read).  g2 rides the c-ring slot
    # freed by chain 6's hand-off; g3's s-slot frees at chain 7's hand-off.
    main_cfg = [("s", s_pool), ("s", s_pool), ("c", c_pool), ("s", s_pool)]
    sts, fots = [], []

    def emit_main(g):
        tag, pool = main_cfg[g]
        st = emit_outproj_main(g, pool, tag)
        sts.append(st)
        fots.append(emit_fin_pre(g, st))

    while pop_av(None):
        if (cons["h"] == 7 and len(sts) < 3
                and cons["kt"] >= 6 + 3 * len(sts)):
            emit_main(len(sts))
    t = tails.pop(6, None)
    if t:
        t()
    tails.pop(7)()
    while len(sts) < 4:
        emit_main(len(sts))
    # round-robin the h7-partial matmuls and merges across the four group
    # tiles so no fin matmul waits on a same-tile merge read
    for sub in range(2):
        for g in range(4):
            emit_fin_mm(g, sts[g], sub)
        for g in range(4):
            emit_fin_merge(g, sts[g], fots[g], sub)


_NC_CACHE = None


def _get_nc():
    global _NC_CACHE
    if _NC_CACHE is None:
        _NC_CACHE = build_nc()
    return _NC_CACHE


def make_in_maps(query, key, value, Wq, bq, Wk, bk, Wv, bv, Wo, bo):
    bf = ml_dtypes.bfloat16
    f = np.float32
    query = np.asarray(query, f)
    key = np.asarray(key, f)
    value = np.asarray(value, f)
    shared = {
        "wq": np.asarray(Wq, f).astype(bf),
        "wk": np.asarray(Wk, f).astype(bf),
        "wv": np.asarray(Wv, f).astype(bf),
        "wo": np.asarray(Wo, f).astype(bf),
        "bq": np.asarray(bq, f).reshape(D, 1),
        "bk": np.asarray(bk, f).reshape(D, 1),
        "bv": np.asarray(bv, f).astype(bf).reshape(1, D),
        "bo": np.asarray(bo, f).astype(bf).reshape(1, D),
    }
    kTs = [np.ascontiguousarray(key[b].T).astype(bf) for b in range(B)]
    vTs = [np.ascontiguousarray(value[b].T).astype(bf) for b in range(B)]
    in_maps = []
    for c in range(8):
        b, half = c // 2, c % 2
        m = dict(shared)
        m["qT"] = np.ascontiguousarray(
            query[b, half * NQ:(half + 1) * NQ, :].T).astype(bf)
        m["kT"] = kTs[b]
        m["vT"] = vTs[b]
        in_maps.append(m)
    return in_maps


def run(inputs, trace=False):
    nc = _get_nc()
    in_maps = make_in_maps(**inputs)
    res = run_bass_kernel_spmd(nc, in_maps, core_ids=list(range(8)), trace=trace)
    out = np.empty((B, N, D), np.float32)
    for c in range(8):
        b, half = c // 2, c % 2
        out[b, half * NQ:(half + 1) * NQ, :] = np.asarray(
            res.results[c]["out"], dtype=np.float32)
    return out, res


def kernel(**inputs):
    out, _ = run(inputs, trace=False)
    return out


# revision 22
# speedup vs baseline: 1.0094x; 1.0012x over previous
# Multi-head attention layer on 8 TRN2 NeuronCores (SPMD, no collectives).
#
# Problem: B=4, N=2048, D=512, H=8 heads (DK=64).
#   out = softmax((q@Wq+bq)(k@Wk+bk)^T / 8) (v@Wv+bv) @ Wo + bo   per (batch, head)
#
# Sharding: core c handles batch b=c//2 and query-row half c%2 (1024 rows).
# K/V projections are recomputed by both cores of a pair (cheap) so there is
# no cross-core communication at all.
#
# v2 over the baseline (229us): the trace showed PE 86% busy (203us) as the
# bottleneck, ACT 62% (147us exp stream), DVE 31%.  Three structural changes:
#
# 1. ROW-TILED S PAIRS.  The S^T matmul contracts over DK=64 — half the PE
#    array.  Head-even (d on partitions 0:64) and head-odd (64:128) S matmuls
#    are emitted adjacently; bass auto-derives tile_position (0,0)/(64,0)
#    (64x128 row-tiling mode T0/T8) and the PE runs them CONCURRENTLY.
#    S time halves: ~55us -> ~27us of PE busy.
#
# 2. EXP SPLIT ACT/DVE.  Per pair-kt two [128,1024] score tiles need exp.
#    Odd-head tiles: exact ACT exp (as before).  Even-head tiles: a one-op
#    DVE "bit trick" exp — y = x*(2^7/ln2)/8 + BIG in fp32, where
#    BIG = 1.5*2^23 + (127<<7) - 8 forces fixed-point rounding so the LOW
#    half-word of each f32 y IS the bf16 bit pattern of exp(x/8) (max rel
#    err ~4.5%, mean 1.5%, washes out under softmax normalization).  The AV
#    matmul reads the bf16 values through a strided bitcast view, so one
#    1.19us DVE op replaces a 1.15us ACT op — both engines stream exps in
#    parallel and exp never gates the PE.
#
# 3. SEQUENTIAL AV CHAINS + SINGLE-COPY HANDOFF.  AV for head-even drains
#    during its own phase (lag ~3 slots), head-odd's pts buffer in SBUF and
#    drain during the next phase, both through ONE PSUM ctx accumulator.
#    At chain end one ACT copy moves [128,1024] (ctx rows + ones-replicated
#    denominator) PSUM->SBUF; the reciprocal/shift/multiply tail runs a few
#    slots later as before.
#
# PSUM: s-ring 3 x [128,1024] (6 banks, shared with projections) + 1 ctx
# (2 banks) = 8 banks exactly.
from contextlib import ExitStack

import numpy as np
import ml_dtypes

import concourse.bass as bass
import concourse.mybir as mybir
import concourse.tile as tile
from concourse import bacc
from concourse.bass_utils import run_bass_kernel_spmd

BF16 = mybir.dt.bfloat16
F32 = mybir.dt.float32
Exp = mybir.ActivationFunctionType.Exp
MULT = mybir.AluOpType.mult
ADD = mybir.AluOpType.add

B, N, D, H = 4, 2048, 512, 8
DK = D // H          # 64
NQ = N // 2          # 1024 query rows per core
NKT = N // 128       # 16 k tiles
NP = H // 2          # 4 head pairs

# exp bit-trick constants: low16(fp32(x*K8 + BIG)) == bf16 bits of exp(x/8)
K8 = float(np.float32(128.0 / np.log(2.0) / 8.0))          # 23.0827
BIG = float(np.float32(1.5 * 2**23 + (127 << 7) - 8))      # fixed-point + bias


def build_nc():
    nc = bacc.Bacc("TRN2", target_bir_lowering=False)

    qT = nc.dram_tensor("qT", (D, NQ), BF16, kind="ExternalInput")
    kT = nc.dram_tensor("kT", (D, N), BF16, kind="ExternalInput")
    vT = nc.dram_tensor("vT", (D, N), BF16, kind="ExternalInput")
    wq = nc.dram_tensor("wq", (D, D), BF16, kind="ExternalInput")
    wk = nc.dram_tensor("wk", (D, D), BF16, kind="ExternalInput")
    wv = nc.dram_tensor("wv", (D, D), BF16, kind="ExternalInput")
    wo = nc.dram_tensor("wo", (D, D), BF16, kind="ExternalInput")
    bq = nc.dram_tensor("bq", (D, 1), F32, kind="ExternalInput")
    bk = nc.dram_tensor("bk", (D, 1), F32, kind="ExternalInput")
    bv = nc.dram_tensor("bv", (1, D), BF16, kind="ExternalInput")
    bo = nc.dram_tensor("bo", (1, D), BF16, kind="ExternalInput")
    out = nc.dram_tensor("out", (NQ, D), BF16, kind="ExternalOutput")

    with tile.TileContext(nc) as tc:
        with ExitStack() as ctx:
            emit(ctx, tc, qT, kT, vT, wq, wk, wv, wo, bq, bk, bv, bo, out)
    nc.compile()
    return nc


def emit(ctx, tc, qT, kT, vT, wq, wk, wv, wo, bq, bk, bv, bo, out):
    nc = tc.nc
    consts = ctx.enter_context(tc.tile_pool(name="consts", bufs=1))
    # odd-head pts (ACT exp, bf16) live up to a full phase before their AV
    p_pool = ctx.enter_context(tc.tile_pool(name="p_pool", bufs=17))
    # even-head DVE-trick tiles (f32, bitcast-read); consumed within ~3 slots
    y_pool = ctx.enter_context(tc.tile_pool(name="y_pool", bufs=5))
    post = ctx.enter_context(tc.tile_pool(name="post", bufs=1))
    outs = ctx.enter_context(tc.tile_pool(name="outs", bufs=4))
    # PSUM: shared 3-deep s-ring (S pairs + projections, 6 banks) + 1 ctx
    # (2 banks) = 8 banks.  (A per-parity 1-deep split was tried and lost
    # ~19us: exp latency ~1.2us needs >=1.5 slots of ring slack.)
    s_pool = ctx.enter_context(tc.tile_pool(name="s_pool", bufs=3, space="PSUM"))
    c_pool = ctx.enter_context(tc.tile_pool(name="c_pool", bufs=1, space="PSUM"))
    dram = ctx.enter_context(tc.tile_pool(name="dram", bufs=1, space="DRAM"))

    # ---- inputs (DMA order = first-use order; big tensors in halves) -------
    def load(name, shape, dt_, src_ap, eng=None):
        t = consts.tile(shape, dt_, name=name)
        (eng or nc.sync).dma_start(out=t, in_=src_ap)
        return t

    def load_halves(name, shape, dt_, dram_t, n, parts=2):
        t = consts.tile(shape, dt_, name=name)
        h = n // parts
        for i in range(parts):
            nc.sync.dma_start(
                out=t[:, :, i * h:(i + 1) * h],
                in_=dram_t[:, i * h:(i + 1) * h].rearrange(
                    "(c p) n -> p c n", p=128))
        return t

    def load_part(t, dram_t, n0, n1):
        nc.sync.dma_start(
            out=t[:, :, n0:n1],
            in_=dram_t[:, n0:n1].rearrange("(c p) n -> p c n", p=128))

    wk_s = load("wk_s", [128, 4, D], BF16, wk[:].rearrange("(c p) d -> p c d", p=128))
    kT_s = consts.tile([128, 4, N], BF16, name="kT_s")
    load_part(kT_s, kT, 0, 512)
    load_part(kT_s, kT, 512, 1024)
    bk_s = load("bk_s", [128, 4, 1], F32, bk[:].rearrange("(c p) o -> p c o", p=128))
    wq_s = load("wq_s", [128, 4, D], BF16, wq[:].rearrange("(c p) d -> p c d", p=128))
    qT_s = load_halves("qT_s", [128, 4, NQ], BF16, qT, NQ)
    bq_s = load("bq_s", [128, 4, 1], F32, bq[:].rearrange("(c p) o -> p c o", p=128))
    wv_s = load("wv_s", [128, 4, D], BF16, wv[:].rearrange("(c p) d -> p c d", p=128))
    bv_bc = load("bv_bc", [128, D], BF16, bv[:].to_broadcast((128, D)))
    load_part(kT_s, kT, 1024, 1536)
    load_part(kT_s, kT, 1536, 2048)
    vT_s = load_halves("vT_s", [128, 4, N], BF16, vT, N, parts=4)
    wo_s = load("wo_s", [128, 4, D], BF16, wo[:].rearrange("(c p) d -> p c d", p=128))
    bo_s = load("bo_s", [1, D], BF16, bo[:])

    ones1 = consts.tile([1, 128], BF16)
    nc.vector.memset(ones1, 1.0)
    ztile = consts.tile([128, 512], BF16)
    nc.vector.memset(ztile, 0.0)

    # tiny dummy exp: pulls the ~2.7us ACT_TABLE_LOAD into the DMA window
    tl = consts.tile([128, 16], F32)
    nc.scalar.activation(tl, ztile[:, 0:16], Exp, scale=1.0)

    KT_s = consts.tile([128, 4, N], BF16)     # K^T, d on partitions
    QT_s = consts.tile([128, 4, NQ], BF16)    # Q^T, d on partitions
    # V with k on partitions; per (kt, head) a 128-wide stationary block:
    # even heads [V(64) | ones(64)], odd heads [ones|V].  The ones half
    # replicates the softmax denominator onto the 64 partitions opposite
    # the ctx rows, so normalization needs no partition broadcast.
    V_s = consts.tile([128, NKT, H, 128], BF16)
    ctxn_s = consts.tile([128, 4, NQ], BF16)  # normalized ctx^T

    V_pairs = V_s[:].rearrange("p t (j par) w -> p t par j w", par=2)
    nc.vector.memset(V_pairs[:, :, 0, :, 64:128], 1.0)  # even heads: ones right
    nc.vector.memset(V_pairs[:, :, 1, :, 0:64], 1.0)    # odd heads: ones left

    # ---- projections --------------------------------------------------------
    def emit_kproj_half(dt, kh):  # one kT half: 8 MMs, one st ring slot
        st = s_pool.tile([128, 1024], F32, tag="s", name="st_k")
        for kc in range(2):
            for cc in range(4):
                nc.tensor.matmul(
                    st[:, kc * 512:(kc + 1) * 512],
                    lhsT=wk_s[:, cc, dt * 128:(dt + 1) * 128],
                    rhs=kT_s[:, cc, kh * 1024 + kc * 512:
                             kh * 1024 + (kc + 1) * 512],
                    start=(cc == 0), stop=(cc == 3))
        nc.scalar.add(
            KT_s[:, dt, kh * 1024:(kh + 1) * 1024], st, bk_s[:, dt, :])

    def emit_qproj(dt):  # 8 MMs, one st ring slot
        st = s_pool.tile([128, 1024], F32, tag="s", name="st_q")
        for qc in range(2):
            for cc in range(4):
                nc.tensor.matmul(
                    st[:, qc * 512:(qc + 1) * 512],
                    lhsT=wq_s[:, cc, dt * 128:(dt + 1) * 128],
                    rhs=qT_s[:, cc, qc * 512:(qc + 1) * 512],
                    start=(cc == 0), stop=(cc == 3))
        nc.scalar.add(QT_s[:, dt, :], st, bq_s[:, dt, :])

    def emit_vproj_pair(g):  # V projection for k tiles 2g, 2g+1
        st = s_pool.tile([128, 1024], F32, tag="s", name="st_v")
        for sub in range(2):
            kt = 2 * g + sub
            sl = st[:, sub * 512:(sub + 1) * 512]
            for cc in range(4):
                nc.tensor.matmul(
                    sl,
                    lhsT=vT_s[:, cc, kt * 128:(kt + 1) * 128],
                    rhs=wv_s[:, cc, :],
                    start=(cc == 0), stop=(cc == 3))
            sl_pairs = sl.rearrange("p (j par w) -> p par j w", par=2, w=64)
            bv_pairs = bv_bc[:].rearrange("p (j par w) -> p par j w",
                                          par=2, w=64)
            vt_pairs = V_s[:, kt].rearrange("p (j par) w -> p par j w", par=2)
            nc.vector.tensor_add(
                vt_pairs[:, 0, :, 0:64], sl_pairs[:, 0], bv_pairs[:, 0])
            nc.vector.tensor_add(
                vt_pairs[:, 1, :, 64:128], sl_pairs[:, 1], bv_pairs[:, 1])

    # ---- attention ----------------------------------------------------------
    def emit_s_pair(p, kt, dve_even):
        """Row-tiled S^T pair for heads (2p, 2p+1) at k-tile kt, plus exp.

        The 4 matmuls alternate partition halves (e,qc0),(o,qc0),(e,qc1),
        (o,qc1): consecutive MMs land on disjoint 64-row PE tiles (T0/T8)
        and run concurrently.  Returns (rhs_e, rhs_o): per-head AV rhs APs.
        """
        st_e = s_pool.tile([128, 1024], F32, tag="s", name="st_e")
        st_o = s_pool.tile([128, 1024], F32, tag="s", name="st_o")
        kcol = slice(kt * 128, (kt + 1) * 128)
        for qc in range(2):
            qs = slice(qc * 512, (qc + 1) * 512)
            nc.tensor.matmul(
                st_e[:, qs], lhsT=KT_s[0:64, p, kcol], rhs=QT_s[0:64, p, qs],
                start=True, stop=True)
            nc.tensor.matmul(
                st_o[:, qs], lhsT=KT_s[64:128, p, kcol], rhs=QT_s[64:128, p, qs],
                start=True, stop=True)
        if dve_even:
            y = y_pool.tile([128, 1024], F32, tag="y", name="y_e")
            nc.vector.tensor_scalar(y, st_e, K8, BIG, op0=MULT, op1=ADD)
            rhs_e = y[:].bitcast(BF16).rearrange(
                "p (n two) -> p n two", two=2)[:, :, 0]
        else:
            pt_e = p_pool.tile([128, 1024], BF16, tag="pe", bufs=2, name="pt_e")
            nc.scalar.activation(pt_e, st_e, Exp, scale=0.125)
            rhs_e = pt_e[:]
        pt_o = p_pool.tile([128, 1024], BF16, tag="p", name="pt_o")
        nc.scalar.activation(pt_o, st_o, Exp, scale=0.125)
        return rhs_e, pt_o[:]

    def emit_av(h, kt, rhs, ctx_ps):
        for qc in range(2):
            nc.tensor.matmul(
                ctx_ps[:, qc * 512:(qc + 1) * 512],
                lhsT=V_s[:, kt, h, :],
                rhs=rhs[:, qc * 512:(qc + 1) * 512],
                start=(kt == 0), stop=(kt == NKT - 1))

    # ---- per-head normalize -------------------------------------------------
    # At chain end one ACT copy stages [128,1024] (ctx half + denominator
    # half) PSUM->SBUF, freeing the single ctx accumulator.  The deferred
    # tail does reciprocal (partitions 0-63 only; partition-shift via
    # SBUF->SBUF DMA on whichever side needs it) and the multiply into ctxn.
    norm7 = {}

    def emit_handoff(h, ctx_ps):
        cc = post.tile([128, NQ], F32, tag="cc", bufs=2, name="cc")
        nc.scalar.copy(out=cc, in_=ctx_ps)
        even = (h % 2 == 0)
        cl, dl = (0, 64) if even else (64, 0)   # ctx / denom partition bases
        if h == H - 1:
            # last head: fold normalize into the output projection — stage
            # raw ctx bf16 (matmul lhsT) + reciprocals transposed onto
            # partitions via a 4KB DRAM bounce.
            def tail():
                rc = post.tile([128, NQ], F32, tag="rc", name="rc")
                nc.vector.reciprocal_approx_fast(out=rc[0:64, :],
                                                 in_=cc[0:64, :])
                ctxc = post.tile([128, NQ], BF16, tag="ctxc7", name="ctxc7")
                nc.vector.tensor_copy(out=ctxc[64:128, :], in_=cc[64:128, :])
                norm7["ctxc"] = ctxc
                dr = dram.tile([1, NQ], F32, tag="dr", name="dr")
                nc.sync.dma_start(out=dr, in_=rc[0:1, :])
                rcol = post.tile([128, 8], F32, tag="rcol", name="rcol")
                nc.sync.dma_start(
                    out=rcol,
                    in_=dr[:].rearrange("o (f p) -> (o p) f", p=128))
                norm7["rcol"] = rcol
            return tail

        def tail():
            dt = h // 2
            d = cc
            if dl != 0:
                den2 = post.tile([128, NQ], F32, tag="shift", name="den2")
                nc.sync.dma_start(out=den2[0:64, :], in_=cc[dl:dl + 64, :])
                d = den2
            rc = post.tile([128, NQ], F32, tag="rc", name="rc")
            nc.vector.reciprocal_approx_fast(out=rc[0:64, :], in_=d[0:64, :])
            if cl != 0:
                rc2 = post.tile([128, NQ], F32, tag="shift", name="rc2")
                nc.sync.dma_start(out=rc2[cl:cl + 64, :], in_=rc[0:64, :])
                rc = rc2
            nc.vector.tensor_mul(ctxn_s[cl:cl + 64, dt, :],
                                 cc[cl:cl + 64, :], rc[cl:cl + 64, :])
        return tail

    # ---- output projection (unchanged from baseline) ------------------------
    def emit_outproj_main(g, pool, tag):
        st = pool.tile([128, 1024], F32, tag=tag, name="st_o")
        for sub in range(2):
            nt = g * 2 + sub
            sl = st[:, sub * 512:(sub + 1) * 512]
            for dc in range(3):
                nc.tensor.matmul(
                    sl,
                    lhsT=ctxn_s[:, dc, nt * 128:(nt + 1) * 128],
                    rhs=wo_s[:, dc, :],
                    start=(dc == 0), stop=False)
            nc.tensor.matmul(
                sl,
                lhsT=ctxn_s[0:64, 3, nt * 128:(nt + 1) * 128],
                rhs=wo_s[0:64, 3, :],
                start=False, stop=False)
            nc.tensor.matmul(sl, lhsT=ones1, rhs=bo_s, start=False, stop=True)
        return st

    def emit_fin_pre(g, st):
        # stage the main sums to SBUF promptly (ACT; frees nothing but gets
        # the copies off the post-fold critical path)
        ots = []
        for sub in range(2):
            sl = st[:, sub * 512:(sub + 1) * 512]
            ot = outs.tile([128, D], BF16, tag="o", bufs=6, name="ot")
            nc.scalar.copy(out=ot, in_=sl)
            ots.append(ot)
        return ots

    def emit_fin_mm(g, st, sub):
        nt = g * 2 + sub
        nc.tensor.matmul(
            st[:, sub * 512:(sub + 1) * 512],
            lhsT=norm7["ctxc"][64:128, nt * 128:(nt + 1) * 128],
            rhs=wo_s[64:128, 3, :],
            start=True, stop=True)

    def emit_fin_merge(g, st, ots, sub):
        # merge = (h7_partial * recip7[n]) + main: the scale rides the ACT
        # copy (per-partition scale AP), the add is a cheap bf16 DVE op —
        # splitting engines keeps the 8 merges off a single serial DVE queue
        nt = g * 2 + sub
        sl = st[:, sub * 512:(sub + 1) * 512]
        tm = outs.tile([128, D], BF16, tag="tm", bufs=3, name="tm")
        nc.scalar.activation(tm, sl, mybir.ActivationFunctionType.Copy,
                             scale=norm7["rcol"][:, nt:nt + 1])
        ot2 = outs.tile([128, D], BF16, tag="o2", bufs=3, name="ot2")
        nc.vector.tensor_add(ot2, tm, ots[sub])
        nc.sync.dma_start(out=out[nt * 128:(nt + 1) * 128, :], in_=ot2)

    # ---- schedule -----------------------------------------------------------
    # warm-up: dummy matmuls during the initial DMA window keep the PE's HAM
    # clock gate at 8/8 until the first projection matmuls are DMA-ready
    wst = s_pool.tile([128, 1024], F32, tag="s", name="wst")
    for i in range(34):
        nc.tensor.matmul(wst[:, 0:512], lhsT=ztile[:, 0:128], rhs=ztile,
                         start=(i == 0), stop=(i == 33))
    emit_kproj_half(0, 0)
    emit_qproj(0)

    # Projection prefetch placement per phase (16 pair-kt slots each).
    # DMA arrival order gates phase-0 placements (kT half 2, all of vT
    # arrive while phase 0 runs).
    def phase_mid(p):
        # K^T/Q^T for phase p+1 must complete during phase p.
        # kproj(p, 1) covers k-tiles 8-15 of phase p, so it can run INSIDE
        # phase p (by slot ~7) — sheds load from the crowded previous phase.
        if p == 0:
            mid = {1: lambda: emit_kproj_half(0, 1),
                   12: lambda: emit_kproj_half(1, 0),
                   14: lambda: emit_qproj(1)}
            for g in range(7):
                mid[3 + g] = (lambda g=g: emit_vproj_pair(g))
            mid[11] = lambda: emit_vproj_pair(7)
            return mid
        if p == 1:
            return {2: lambda: emit_kproj_half(1, 1),
                    6: lambda: emit_kproj_half(2, 0),
                    10: lambda: emit_qproj(2)}
        if p == 2:
            return {2: lambda: emit_kproj_half(2, 1),
                    6: lambda: emit_kproj_half(3, 0),
                    10: lambda: emit_qproj(3)}
        return {3: lambda: emit_kproj_half(3, 1)}

    # AV consumption: strictly chain-sequential (head 0 fully, then 1, ...)
    # through the single ctx accumulator.  prod[h][kt] = (rhs, produced_slot);
    # an entry is poppable LAG slots after production (so the PE never
    # head-blocks waiting on its exp).  Hand-off at kt==15, tail deferred to
    # the next chain's kt==4.
    LAG = 2
    prod = {h: {} for h in range(H)}
    cons = {"h": 0, "kt": 0, "n": 0, "ctx": None}
    tails = {}
    n_prod = [0]

    def next_entry():
        if cons["h"] >= H:
            return None
        return prod[cons["h"]].get(cons["kt"])

    def pop_av(now_slot):
        e = next_entry()
        if e is None or (now_slot is not None and now_slot < e[1] + LAG):
            return False
        h, kt = cons["h"], cons["kt"]
        if kt == 0:
            # last chain: take a (by-then free) s-ring slot so its first AV
            # doesn't serialize behind the previous chain's hand-off copy
            pool, tag = (s_pool, "s") if h == H - 1 else (c_pool, "c")
            cons["ctx"] = pool.tile([128, 1024], F32, tag=tag, name="ctx_ps")
        emit_av(h, kt, e[0], cons["ctx"])
        cons["n"] += 1
        if kt == 4 and h >= 1:
            t = tails.pop(h - 1, None)
            if t:
                t()
        if kt == NKT - 1:
            tails[h] = emit_handoff(h, cons["ctx"])
            cons["h"] += 1
            cons["kt"] = 0
        else:
            cons["kt"] += 1
        return True

    for p in range(NP):
        mid = phase_mid(p)
        for kt in range(NKT):
            slot = p * NKT + kt
            if kt in mid:
                mid[kt]()
            rhs_e, rhs_o = emit_s_pair(p, kt, True)
            prod[2 * p][kt] = (rhs_e, slot)
            prod[2 * p + 1][kt] = (rhs_o, slot)
            n_prod[0] += 2
            target = max(6, 11 - kt)
            while n_prod[0] - cons["n"] > target and pop_av(slot):
                pass
    # Final drain: chain 6 remainder + chain 7, with outproj mains g0-g2
    # interleaved into the AV stream (tail(6) fires at chain-7 kt4,
    # unblocking the h6 half-column they read).  g2 rides the c-ring slot
    # freed by chain 6's hand-off; g3's s-slot frees at chain 7's hand-off.
    main_cfg = [("s", s_pool), ("s", s_pool), ("c", c_pool), ("s", s_pool)]
    sts, fots = [], []

    def emit_main(g):
        tag, pool = main_cfg[g]
        st = emit_outproj_main(g, pool, tag)
        sts.append(st)
        fots.append(emit_fin_pre(g, st))

    while pop_av(None):
        if (cons["h"] == 7 and len(sts) < 3
                and cons["kt"] >= 6 + 3 * len(sts)):
            emit_main(len(sts))
    t = tails.pop(6, None)
    if t:
        t()
    tails.pop(7)()
    while len(sts) < 4:
        emit_main(len(sts))
    # round-robin the h7-partial matmuls and merges across the four group
    # tiles so no fin matmul waits on a same-tile merge read
    for sub in range(2):
        for g in range(4):
            emit_fin_mm(g, sts[g], sub)
        for g in range(4):
            emit_fin_merge(g, sts[g], fots[g], sub)


_NC_CACHE = None


def _get_nc():
    global _NC_CACHE
    if _NC_CACHE is None:
        _NC_CACHE = build_nc()
    return _NC_CACHE


def make_in_maps(query, key, value, Wq, bq, Wk, bk, Wv, bv, Wo, bo):
    bf = ml_dtypes.bfloat16
    f = np.float32
    query = np.asarray(query, f)
    key = np.asarray(key, f)
    value = np.asarray(value, f)
    shared = {
        "wq": np.asarray(Wq, f).astype(bf),
        "wk": np.asarray(Wk, f).astype(bf),
        "wv": np.asarray(Wv, f).astype(bf),
        "wo": np.asarray(Wo, f).astype(bf),
        "bq": np.asarray(bq, f).reshape(D, 1),
        "bk": np.asarray(bk, f).reshape(D, 1),
        "bv": np.asarray(bv, f).astype(bf).reshape(1, D),
        "bo": np.asarray(bo, f).astype(bf).reshape(1, D),
    }
    kTs = [np.ascontiguousarray(key[b].T).astype(bf) for b in range(B)]
    vTs = [np.ascontiguousarray(value[b].T).astype(bf) for b in range(B)]
    in_maps = []
    for c in range(8):
        b, half = c // 2, c % 2
        m = dict(shared)
        m["qT"] = np.ascontiguousarray(
            query[b, half * NQ:(half + 1) * NQ, :].T).astype(bf)
        m["kT"] = kTs[b]
        m["vT"] = vTs[b]
        in_maps.append(m)
    return in_maps


def run(inputs, trace=False):
    nc = _get_nc()
    in_maps = make_in_maps(**inputs)
    res = run_bass_kernel_spmd(nc, in_maps, core_ids=list(range(8)), trace=trace)
    out = np.empty((B, N, D), np.float32)
    for c in range(8):
        b, half = c // 2, c % 2
        out[b, half * NQ:(half + 1) * NQ, :] = np.asarray(
            res.results[c]["out"], dtype=np.float32)
    return out, res


def kernel(**inputs):
    out, _ = run(inputs, trace=False)
    return out


# revision 24
# speedup vs baseline: 1.0238x; 1.0142x over previous
# Multi-head attention layer on 8 TRN2 NeuronCores (SPMD, no collectives).
#
# Problem: B=4, N=2048, D=512, H=8 heads (DK=64).
#   out = softmax((q@Wq+bq)(k@Wk+bk)^T / 8) (v@Wv+bv) @ Wo + bo   per (batch, head)
#
# Sharding: core c handles batch b=c//2 and query-row half c%2 (1024 rows).
# K/V projections are recomputed by both cores of a pair (cheap) so there is
# no cross-core communication at all.
#
# v2 over the baseline (229us): the trace showed PE 86% busy (203us) as the
# bottleneck, ACT 62% (147us exp stream), DVE 31%.  Three structural changes:
#
# 1. ROW-TILED S PAIRS.  The S^T matmul contracts over DK=64 — half the PE
#    array.  Head-even (d on partitions 0:64) and head-odd (64:128) S matmuls
#    are emitted adjacently; bass auto-derives tile_position (0,0)/(64,0)
#    (64x128 row-tiling mode T0/T8) and the PE runs them CONCURRENTLY.
#    S time halves: ~55us -> ~27us of PE busy.
#
# 2. EXP SPLIT ACT/DVE.  Per pair-kt two [128,1024] score tiles need exp.
#    Odd-head tiles: exact ACT exp (as before).  Even-head tiles: a one-op
#    DVE "bit trick" exp — y = x*(2^7/ln2)/8 + BIG in fp32, where
#    BIG = 1.5*2^23 + (127<<7) - 8 forces fixed-point rounding so the LOW
#    half-word of each f32 y IS the bf16 bit pattern of exp(x/8) (max rel
#    err ~4.5%, mean 1.5%, washes out under softmax normalization).  The AV
#    matmul reads the bf16 values through a strided bitcast view, so one
#    1.19us DVE op replaces a 1.15us ACT op — both engines stream exps in
#    parallel and exp never gates the PE.
#
# 3. SEQUENTIAL AV CHAINS + SINGLE-COPY HANDOFF.  AV for head-even drains
#    during its own phase (lag ~3 slots), head-odd's pts buffer in SBUF and
#    drain during the next phase, both through ONE PSUM ctx accumulator.
#    At chain end one ACT copy moves [128,1024] (ctx rows + ones-replicated
#    denominator) PSUM->SBUF; the reciprocal/shift/multiply tail runs a few
#    slots later as before.
#
# PSUM: s-ring 3 x [128,1024] (6 banks, shared with projections) + 1 ctx
# (2 banks) = 8 banks exactly.
from contextlib import ExitStack

import numpy as np
import ml_dtypes

import concourse.bass as bass
import concourse.mybir as mybir
import concourse.tile as tile
from concourse import bacc
from concourse.bass_utils import run_bass_kernel_spmd

BF16 = mybir.dt.bfloat16
F32 = mybir.dt.float32
Exp = mybir.ActivationFunctionType.Exp
MULT = mybir.AluOpType.mult
ADD = mybir.AluOpType.add

B, N, D, H = 4, 2048, 512, 8
DK = D // H          # 64
NQ = N // 2          # 1024 query rows per core
NKT = N // 128       # 16 k tiles
NP = H // 2          # 4 head pairs

# exp bit-trick constants: low16(fp32(x*K8 + BIG)) == bf16 bits of exp(x/8)
K8 = float(np.float32(128.0 / np.log(2.0) / 8.0))          # 23.0827
BIG = float(np.float32(1.5 * 2**23 + (127 << 7) - 8))      # fixed-point + bias


def build_nc():
    nc = bacc.Bacc("TRN2", target_bir_lowering=False)

    qT = nc.dram_tensor("qT", (D, NQ), BF16, kind="ExternalInput")
    kT = nc.dram_tensor("kT", (D, N), BF16, kind="ExternalInput")
    vT = nc.dram_tensor("vT", (D, N), BF16, kind="ExternalInput")
    wq = nc.dram_tensor("wq", (D, D), BF16, kind="ExternalInput")
    wk = nc.dram_tensor("wk", (D, D), BF16, kind="ExternalInput")
    wv = nc.dram_tensor("wv", (D, D), BF16, kind="ExternalInput")
    wo = nc.dram_tensor("wo", (D, D), BF16, kind="ExternalInput")
    bq = nc.dram_tensor("bq", (D, 1), F32, kind="ExternalInput")
    bk = nc.dram_tensor("bk", (D, 1), F32, kind="ExternalInput")
    bv = nc.dram_tensor("bv", (1, D), BF16, kind="ExternalInput")
    bo = nc.dram_tensor("bo", (1, D), BF16, kind="ExternalInput")
    out = nc.dram_tensor("out", (NQ, D), BF16, kind="ExternalOutput")

    with tile.TileContext(nc) as tc:
        with ExitStack() as ctx:
            emit(ctx, tc, qT, kT, vT, wq, wk, wv, wo, bq, bk, bv, bo, out)
    nc.compile()
    return nc


def emit(ctx, tc, qT, kT, vT, wq, wk, wv, wo, bq, bk, bv, bo, out):
    nc = tc.nc
    consts = ctx.enter_context(tc.tile_pool(name="consts", bufs=1))
    # odd-head pts (ACT exp, bf16) live up to a full phase before their AV
    p_pool = ctx.enter_context(tc.tile_pool(name="p_pool", bufs=17))
    # even-head DVE-trick tiles (f32, bitcast-read); consumed within ~3 slots
    y_pool = ctx.enter_context(tc.tile_pool(name="y_pool", bufs=4))
    post = ctx.enter_context(tc.tile_pool(name="post", bufs=1))
    outs = ctx.enter_context(tc.tile_pool(name="outs", bufs=4))
    # PSUM: shared 3-deep s-ring (S pairs + projections, 6 banks) + 1 ctx
    # (2 banks) = 8 banks.  (A per-parity 1-deep split was tried and lost
    # ~19us: exp latency ~1.2us needs >=1.5 slots of ring slack.)
    s_pool = ctx.enter_context(tc.tile_pool(name="s_pool", bufs=3, space="PSUM"))
    c_pool = ctx.enter_context(tc.tile_pool(name="c_pool", bufs=1, space="PSUM"))
    dram = ctx.enter_context(tc.tile_pool(name="dram", bufs=1, space="DRAM"))

    # ---- inputs (DMA order = first-use order; big tensors in halves) -------
    def load(name, shape, dt_, src_ap, eng=None):
        t = consts.tile(shape, dt_, name=name)
        (eng or nc.sync).dma_start(out=t, in_=src_ap)
        return t

    def load_halves(name, shape, dt_, dram_t, n, parts=2):
        t = consts.tile(shape, dt_, name=name)
        h = n // parts
        for i in range(parts):
            nc.sync.dma_start(
                out=t[:, :, i * h:(i + 1) * h],
                in_=dram_t[:, i * h:(i + 1) * h].rearrange(
                    "(c p) n -> p c n", p=128))
        return t

    def load_part(t, dram_t, n0, n1):
        nc.sync.dma_start(
            out=t[:, :, n0:n1],
            in_=dram_t[:, n0:n1].rearrange("(c p) n -> p c n", p=128))

    wk_s = load("wk_s", [128, 4, D], BF16, wk[:].rearrange("(c p) d -> p c d", p=128))
    kT_s = consts.tile([128, 4, N], BF16, name="kT_s")
    load_part(kT_s, kT, 0, 512)
    load_part(kT_s, kT, 512, 1024)
    bk_s = load("bk_s", [128, 4, 1], F32, bk[:].rearrange("(c p) o -> p c o", p=128))
    wq_s = load("wq_s", [128, 4, D], BF16, wq[:].rearrange("(c p) d -> p c d", p=128))
    qT_s = load_halves("qT_s", [128, 4, NQ], BF16, qT, NQ)
    bq_s = load("bq_s", [128, 4, 1], F32, bq[:].rearrange("(c p) o -> p c o", p=128))
    wv_s = load("wv_s", [128, 4, D], BF16, wv[:].rearrange("(c p) d -> p c d", p=128))
    bv_bc = load("bv_bc", [128, D], BF16, bv[:].to_broadcast((128, D)))
    load_part(kT_s, kT, 1024, 1536)
    load_part(kT_s, kT, 1536, 2048)
    vT_s = load_halves("vT_s", [128, 4, N], BF16, vT, N, parts=4)
    wo_s = load("wo_s", [128, 4, D], BF16, wo[:].rearrange("(c p) d -> p c d", p=128))
    bo_s = load("bo_s", [1, D], BF16, bo[:])

    ones1 = consts.tile([1, 128], BF16)
    nc.vector.memset(ones1, 1.0)
    ztile = consts.tile([128, 512], BF16)
    nc.vector.memset(ztile, 0.0)

    # tiny dummy exp: pulls the ~2.7us ACT_TABLE_LOAD into the DMA window
    tl = consts.tile([128, 16], F32)
    nc.scalar.activation(tl, ztile[:, 0:16], Exp, scale=1.0)

    KT_s = consts.tile([128, 4, N], BF16)     # K^T, d on partitions
    QT_s = consts.tile([128, 4, NQ], BF16)    # Q^T, d on partitions
    # V with k on partitions; per (kt, head) a 128-wide stationary block:
    # even heads [V(64) | ones(64)], odd heads [ones|V].  The ones half
    # replicates the softmax denominator onto the 64 partitions opposite
    # the ctx rows, so normalization needs no partition broadcast.
    V_s = consts.tile([128, NKT, H, 128], BF16)
    ctxn_s = consts.tile([128, 4, NQ], BF16)  # normalized ctx^T

    V_pairs = V_s[:].rearrange("p t (j par) w -> p t par j w", par=2)
    nc.vector.memset(V_pairs[:, :, 0, :, 64:128], 1.0)  # even heads: ones right
    nc.vector.memset(V_pairs[:, :, 1, :, 0:64], 1.0)    # odd heads: ones left

    # ---- projections --------------------------------------------------------
    def emit_kproj_half(dt, kh):  # one kT half: 8 MMs, one st ring slot
        st = s_pool.tile([128, 1024], F32, tag="s", name="st_k")
        for kc in range(2):
            for cc in range(4):
                nc.tensor.matmul(
                    st[:, kc * 512:(kc + 1) * 512],
                    lhsT=wk_s[:, cc, dt * 128:(dt + 1) * 128],
                    rhs=kT_s[:, cc, kh * 1024 + kc * 512:
                             kh * 1024 + (kc + 1) * 512],
                    start=(cc == 0), stop=(cc == 3))
        nc.scalar.add(
            KT_s[:, dt, kh * 1024:(kh + 1) * 1024], st, bk_s[:, dt, :])

    def emit_qproj(dt):  # 8 MMs, one st ring slot
        st = s_pool.tile([128, 1024], F32, tag="s", name="st_q")
        for qc in range(2):
            for cc in range(4):
                nc.tensor.matmul(
                    st[:, qc * 512:(qc + 1) * 512],
                    lhsT=wq_s[:, cc, dt * 128:(dt + 1) * 128],
                    rhs=qT_s[:, cc, qc * 512:(qc + 1) * 512],
                    start=(cc == 0), stop=(cc == 3))
        nc.scalar.add(QT_s[:, dt, :], st, bq_s[:, dt, :])

    def emit_vproj_pair(g):  # V projection for k tiles 2g, 2g+1
        st = s_pool.tile([128, 1024], F32, tag="s", name="st_v")
        for sub in range(2):
            kt = 2 * g + sub
            sl = st[:, sub * 512:(sub + 1) * 512]
            for cc in range(4):
                nc.tensor.matmul(
                    sl,
                    lhsT=vT_s[:, cc, kt * 128:(kt + 1) * 128],
                    rhs=wv_s[:, cc, :],
                    start=(cc == 0), stop=(cc == 3))
            sl_pairs = sl.rearrange("p (j par w) -> p par j w", par=2, w=64)
            bv_pairs = bv_bc[:].rearrange("p (j par w) -> p par j w",
                                          par=2, w=64)
            vt_pairs = V_s[:, kt].rearrange("p (j par) w -> p par j w", par=2)
            nc.vector.tensor_add(
                vt_pairs[:, 0, :, 0:64], sl_pairs[:, 0], bv_pairs[:, 0])
            nc.vector.tensor_add(
                vt_pairs[:, 1, :, 64:128], sl_pairs[:, 1], bv_pairs[:, 1])

    # ---- attention ----------------------------------------------------------
    def emit_s_pair(p, kt, dve_even):
        """Row-tiled S^T pair for heads (2p, 2p+1) at k-tile kt, plus exp.

        The 4 matmuls alternate partition halves (e,qc0),(o,qc0),(e,qc1),
        (o,qc1): consecutive MMs land on disjoint 64-row PE tiles (T0/T8)
        and run concurrently.  Returns (rhs_e, rhs_o): per-head AV rhs APs.
        """
        st_e = s_pool.tile([128, 1024], F32, tag="s", name="st_e")
        st_o = s_pool.tile([128, 1024], F32, tag="s", name="st_o")
        kcol = slice(kt * 128, (kt + 1) * 128)
        for qc in range(2):
            qs = slice(qc * 512, (qc + 1) * 512)
            nc.tensor.matmul(
                st_e[:, qs], lhsT=KT_s[0:64, p, kcol], rhs=QT_s[0:64, p, qs],
                start=True, stop=True)
            nc.tensor.matmul(
                st_o[:, qs], lhsT=KT_s[64:128, p, kcol], rhs=QT_s[64:128, p, qs],
                start=True, stop=True)
        if dve_even:
            y = y_pool.tile([128, 1024], F32, tag="y", name="y_e")
            nc.vector.tensor_scalar(y, st_e, K8, BIG, op0=MULT, op1=ADD)
            rhs_e = y[:].bitcast(BF16).rearrange(
                "p (n two) -> p n two", two=2)[:, :, 0]
        else:
            pt_e = p_pool.tile([128, 1024], BF16, tag="pe", bufs=2, name="pt_e")
            nc.scalar.activation(pt_e, st_e, Exp, scale=0.125)
            rhs_e = pt_e[:]
        pt_o = p_pool.tile([128, 1024], BF16, tag="p", name="pt_o")
        nc.scalar.activation(pt_o, st_o, Exp, scale=0.125)
        return rhs_e, pt_o[:]

    def emit_av(h, kt, rhs, ctx_ps):
        for qc in range(2):
            nc.tensor.matmul(
                ctx_ps[:, qc * 512:(qc + 1) * 512],
                lhsT=V_s[:, kt, h, :],
                rhs=rhs[:, qc * 512:(qc + 1) * 512],
                start=(kt == 0), stop=(kt == NKT - 1))

    # ---- per-head normalize -------------------------------------------------
    # At chain end one ACT copy stages [128,1024] (ctx half + denominator
    # half) PSUM->SBUF, freeing the single ctx accumulator.  The deferred
    # tail does reciprocal (partitions 0-63 only; partition-shift via
    # SBUF->SBUF DMA on whichever side needs it) and the multiply into ctxn.
    norm7 = {}

    def emit_handoff(h, ctx_ps):
        cc = post.tile([128, NQ], F32, tag="cc", bufs=2, name="cc")
        nc.scalar.copy(out=cc, in_=ctx_ps)
        even = (h % 2 == 0)
        cl, dl = (0, 64) if even else (64, 0)   # ctx / denom partition bases
        if h == H - 1:
            # last head: fold normalize into the output projection — stage
            # raw ctx bf16 (matmul lhsT) + reciprocals transposed onto
            # partitions via a 4KB DRAM bounce.
            def tail():
                rc = post.tile([128, NQ], F32, tag="rc", name="rc")
                nc.vector.reciprocal_approx_fast(out=rc[0:64, :],
                                                 in_=cc[0:64, :])
                ctxc = post.tile([128, NQ], BF16, tag="ctxc7", name="ctxc7")
                nc.vector.tensor_copy(out=ctxc[64:128, :], in_=cc[64:128, :])
                norm7["ctxc"] = ctxc
                dr = dram.tile([1, NQ], F32, tag="dr", name="dr")
                nc.sync.dma_start(out=dr, in_=rc[0:1, :])
                rcol = post.tile([128, 8], F32, tag="rcol", name="rcol")
                nc.sync.dma_start(
                    out=rcol,
                    in_=dr[:].rearrange("o (f p) -> (o p) f", p=128))
                norm7["rcol"] = rcol
            return tail

        def tail():
            dt = h // 2
            d = cc
            if dl != 0:
                den2 = post.tile([128, NQ], F32, tag="shift", name="den2")
                nc.sync.dma_start(out=den2[0:64, :], in_=cc[dl:dl + 64, :])
                d = den2
            rc = post.tile([128, NQ], F32, tag="rc", name="rc")
            nc.vector.reciprocal_approx_fast(out=rc[0:64, :], in_=d[0:64, :])
            if cl != 0:
                rc2 = post.tile([128, NQ], F32, tag="shift", name="rc2")
                nc.sync.dma_start(out=rc2[cl:cl + 64, :], in_=rc[0:64, :])
                rc = rc2
            nc.vector.tensor_mul(ctxn_s[cl:cl + 64, dt, :],
                                 cc[cl:cl + 64, :], rc[cl:cl + 64, :])
        return tail

    # ---- output projection (unchanged from baseline) ------------------------
    def emit_outproj_main(g, pool, tag):
        st = pool.tile([128, 1024], F32, tag=tag, name="st_o")
        for sub in range(2):
            nt = g * 2 + sub
            sl = st[:, sub * 512:(sub + 1) * 512]
            for dc in range(3):
                nc.tensor.matmul(
                    sl,
                    lhsT=ctxn_s[:, dc, nt * 128:(nt + 1) * 128],
                    rhs=wo_s[:, dc, :],
                    start=(dc == 0), stop=False)
            nc.tensor.matmul(
                sl,
                lhsT=ctxn_s[0:64, 3, nt * 128:(nt + 1) * 128],
                rhs=wo_s[0:64, 3, :],
                start=False, stop=False)
            nc.tensor.matmul(sl, lhsT=ones1, rhs=bo_s, start=False, stop=True)
        return st

    def emit_fin_pre(g, st):
        # stage the main sums to SBUF promptly (ACT; frees nothing but gets
        # the copies off the post-fold critical path)
        ots = []
        for sub in range(2):
            sl = st[:, sub * 512:(sub + 1) * 512]
            ot = outs.tile([128, D], BF16, tag="o", bufs=6, name="ot")
            nc.scalar.copy(out=ot, in_=sl)
            ots.append(ot)
        return ots

    def emit_fin_mm(g, st, sub):
        nt = g * 2 + sub
        nc.tensor.matmul(
            st[:, sub * 512:(sub + 1) * 512],
            lhsT=norm7["ctxc"][64:128, nt * 128:(nt + 1) * 128],
            rhs=wo_s[64:128, 3, :],
            start=True, stop=True)

    def emit_fin_merge(g, st, ots, sub):
        # merge = (h7_partial * recip7[n]) + main: the scale rides the ACT
        # copy (per-partition scale AP), the add is a cheap bf16 DVE op —
        # splitting engines keeps the 8 merges off a single serial DVE queue
        nt = g * 2 + sub
        sl = st[:, sub * 512:(sub + 1) * 512]
        tm = outs.tile([128, D], BF16, tag="tm", bufs=3, name="tm")
        nc.scalar.activation(tm, sl, mybir.ActivationFunctionType.Copy,
                             scale=norm7["rcol"][:, nt:nt + 1])
        ot2 = outs.tile([128, D], BF16, tag="o2", bufs=3, name="ot2")
        nc.vector.tensor_add(ot2, tm, ots[sub])
        nc.sync.dma_start(out=out[nt * 128:(nt + 1) * 128, :], in_=ot2)

    # ---- schedule -----------------------------------------------------------
    # warm-up: dummy matmuls during the initial DMA window keep the PE's HAM
    # clock gate at 8/8 until the first projection matmuls are DMA-ready
    wst = s_pool.tile([128, 1024], F32, tag="s", name="wst")
    for i in range(12):
        nc.tensor.matmul(wst[:, 0:512], lhsT=ztile[:, 0:128], rhs=ztile,
                         start=(i == 0), stop=(i == 11))
    # startup: interleave the first K/Q projection halves with per-half
    # biases on alternating engines — S(0,0) needs only kc0+qc0+qc1, so the
    # serial [proj -> bias -> proj -> bias] ACT chain is cut down
    stk = s_pool.tile([128, 1024], F32, tag="s", name="stk")
    stq = s_pool.tile([128, 1024], F32, tag="s", name="stq")
    for half in range(2):
        for cc in range(4):
            nc.tensor.matmul(
                stk[:, half * 512:(half + 1) * 512],
                lhsT=wk_s[:, cc, 0:128],
                rhs=kT_s[:, cc, half * 512:(half + 1) * 512],
                start=(cc == 0), stop=(cc == 3))
        nc.vector.tensor_scalar_add(
            KT_s[:, 0, half * 512:(half + 1) * 512],
            stk[:, half * 512:(half + 1) * 512], bk_s[:, 0, :])
        for cc in range(4):
            nc.tensor.matmul(
                stq[:, half * 512:(half + 1) * 512],
                lhsT=wq_s[:, cc, 0:128],
                rhs=qT_s[:, cc, half * 512:(half + 1) * 512],
                start=(cc == 0), stop=(cc == 3))
        nc.scalar.add(QT_s[:, 0, half * 512:(half + 1) * 512],
                      stq[:, half * 512:(half + 1) * 512], bq_s[:, 0, :])

    # Projection prefetch placement per phase (16 pair-kt slots each).
    # DMA arrival order gates phase-0 placements (kT half 2, all of vT
    # arrive while phase 0 runs).
    def phase_mid(p):
        # K^T/Q^T for phase p+1 must complete during phase p.
        # kproj(p, 1) covers k-tiles 8-15 of phase p, so it can run INSIDE
        # phase p (by slot ~7) — sheds load from the crowded previous phase.
        if p == 0:
            mid = {1: lambda: emit_kproj_half(0, 1),
                   12: lambda: emit_kproj_half(1, 0),
                   14: lambda: emit_qproj(1)}
            for g in range(7):
                mid[3 + g] = (lambda g=g: emit_vproj_pair(g))
            mid[11] = lambda: emit_vproj_pair(7)
            return mid
        if p == 1:
            return {2: lambda: emit_kproj_half(1, 1),
                    6: lambda: emit_kproj_half(2, 0),
                    10: lambda: emit_qproj(2)}
        if p == 2:
            return {2: lambda: emit_kproj_half(2, 1),
                    6: lambda: emit_kproj_half(3, 0),
                    10: lambda: emit_qproj(3)}
        return {3: lambda: emit_kproj_half(3, 1)}

    # AV consumption: strictly chain-sequential (head 0 fully, then 1, ...)
    # through the single ctx accumulator.  prod[h][kt] = (rhs, produced_slot);
    # an entry is poppable LAG slots after production (so the PE never
    # head-blocks waiting on its exp).  Hand-off at kt==15, tail deferred to
    # the next chain's kt==4.
    LAG = 2
    prod = {h: {} for h in range(H)}
    cons = {"h": 0, "kt": 0, "n": 0, "ctx": None}
    tails = {}
    n_prod = [0]

    def next_entry():
        if cons["h"] >= H:
            return None
        return prod[cons["h"]].get(cons["kt"])

    def pop_av(now_slot):
        e = next_entry()
        if e is None or (now_slot is not None and now_slot < e[1] + LAG):
            return False
        h, kt = cons["h"], cons["kt"]
        if kt == 0:
            # last chain: take a (by-then free) s-ring slot so its first AV
            # doesn't serialize behind the previous chain's hand-off copy
            pool, tag = (s_pool, "s") if h == H - 1 else (c_pool, "c")
            cons["ctx"] = pool.tile([128, 1024], F32, tag=tag, name="ctx_ps")
        emit_av(h, kt, e[0], cons["ctx"])
        cons["n"] += 1
        if kt == 4 and h >= 1:
            t = tails.pop(h - 1, None)
            if t:
                t()
        if kt == NKT - 1:
            tails[h] = emit_handoff(h, cons["ctx"])
            cons["h"] += 1
            cons["kt"] = 0
        else:
            cons["kt"] += 1
        return True

    for p in range(NP):
        mid = phase_mid(p)
        for kt in range(NKT):
            slot = p * NKT + kt
            if kt in mid:
                mid[kt]()
            dve_even = not (p == 0 and kt in (0, 2, 5, 7, 10, 12, 15))
            rhs_e, rhs_o = emit_s_pair(p, kt, dve_even)
            prod[2 * p][kt] = (rhs_e, slot)
            prod[2 * p + 1][kt] = (rhs_o, slot)
            n_prod[0] += 2
            target = max(6, 11 - kt)
            while n_prod[0] - cons["n"] > target and pop_av(slot):
                pass
    # Final drain: chain 6 remainder + chain 7, with outproj mains g0-g2
    # interleaved into the AV stream (tail(6) fires at chain-7 kt4,
    # unblocking the h6 half-column they read).  g2 rides the c-ring slot
    # freed by chain 6's hand-off; g3's s-slot frees at chain 7's hand-off.
    main_cfg = [("s", s_pool), ("s", s_pool), ("c", c_pool), ("s", s_pool)]
    sts, fots = [], []

    def emit_main(g):
        tag, pool = main_cfg[g]
        st = emit_outproj_main(g, pool, tag)
        sts.append(st)
        fots.append(emit_fin_pre(g, st))

    while pop_av(None):
        if (cons["h"] == 7 and len(sts) < 3
                and cons["kt"] >= 6 + 3 * len(sts)):
            emit_main(len(sts))
    t = tails.pop(6, None)
    if t:
        t()
    tails.pop(7)()
    while len(sts) < 4:
        emit_main(len(sts))
    # round-robin the h7-partial matmuls and merges across the four group
    # tiles so no fin matmul waits on a same-tile merge read
    for sub in range(2):
        for g in range(4):
            emit_fin_mm(g, sts[g], sub)
        for g in range(4):
            emit_fin_merge(g, sts[g], fots[g], sub)


_NC_CACHE = None


def _get_nc():
    global _NC_CACHE
    if _NC_CACHE is None:
        _NC_CACHE = build_nc()
    return _NC_CACHE


def make_in_maps(query, key, value, Wq, bq, Wk, bk, Wv, bv, Wo, bo):
    bf = ml_dtypes.bfloat16
    f = np.float32
    query = np.asarray(query, f)
    key = np.asarray(key, f)
    value = np.asarray(value, f)
    shared = {
        "wq": np.asarray(Wq, f).astype(bf),
        "wk": np.asarray(Wk, f).astype(bf),
        "wv": np.asarray(Wv, f).astype(bf),
        "wo": np.asarray(Wo, f).astype(bf),
        "bq": np.asarray(bq, f).reshape(D, 1),
        "bk": np.asarray(bk, f).reshape(D, 1),
        "bv": np.asarray(bv, f).astype(bf).reshape(1, D),
        "bo": np.asarray(bo, f).astype(bf).reshape(1, D),
    }
    kTs = [np.ascontiguousarray(key[b].T).astype(bf) for b in range(B)]
    vTs = [np.ascontiguousarray(value[b].T).astype(bf) for b in range(B)]
    in_maps = []
    for c in range(8):
        b, half = c // 2, c % 2
        m = dict(shared)
        m["qT"] = np.ascontiguousarray(
            query[b, half * NQ:(half + 1) * NQ, :].T).astype(bf)
        m["kT"] = kTs[b]
        m["vT"] = vTs[b]
        in_maps.append(m)
    return in_maps


def run(inputs, trace=False):
    nc = _get_nc()
    in_maps = make_in_maps(**inputs)
    res = run_bass_kernel_spmd(nc, in_maps, core_ids=list(range(8)), trace=trace)
    out = np.empty((B, N, D), np.float32)
    for c in range(8):
        b, half = c // 2, c % 2
        out[b, half * NQ:(half + 1) * NQ, :] = np.asarray(
            res.results[c]["out"], dtype=np.float32)
    return out, res


def kernel(**inputs):
    out, _ = run(inputs, trace=False)
    return out


# revision 25
# speedup vs baseline: 1.0270x; 1.0032x over previous
# Multi-head attention layer on 8 TRN2 NeuronCores (SPMD, no collectives).
#
# Problem: B=4, N=2048, D=512, H=8 heads (DK=64).
#   out = softmax((q@Wq+bq)(k@Wk+bk)^T / 8) (v@Wv+bv) @ Wo + bo   per (batch, head)
#
# Sharding: core c handles batch b=c//2 and query-row half c%2 (1024 rows).
# K/V projections are recomputed by both cores of a pair (cheap) so there is
# no cross-core communication at all.
#
# v2 over the baseline (229us): the trace showed PE 86% busy (203us) as the
# bottleneck, ACT 62% (147us exp stream), DVE 31%.  Three structural changes:
#
# 1. ROW-TILED S PAIRS.  The S^T matmul contracts over DK=64 — half the PE
#    array.  Head-even (d on partitions 0:64) and head-odd (64:128) S matmuls
#    are emitted adjacently; bass auto-derives tile_position (0,0)/(64,0)
#    (64x128 row-tiling mode T0/T8) and the PE runs them CONCURRENTLY.
#    S time halves: ~55us -> ~27us of PE busy.
#
# 2. EXP SPLIT ACT/DVE.  Per pair-kt two [128,1024] score tiles need exp.
#    Odd-head tiles: exact ACT exp (as before).  Even-head tiles: a one-op
#    DVE "bit trick" exp — y = x*(2^7/ln2)/8 + BIG in fp32, where
#    BIG = 1.5*2^23 + (127<<7) - 8 forces fixed-point rounding so the LOW
#    half-word of each f32 y IS the bf16 bit pattern of exp(x/8) (max rel
#    err ~4.5%, mean 1.5%, washes out under softmax normalization).  The AV
#    matmul reads the bf16 values through a strided bitcast view, so one
#    1.19us DVE op replaces a 1.15us ACT op — both engines stream exps in
#    parallel and exp never gates the PE.
#
# 3. SEQUENTIAL AV CHAINS + SINGLE-COPY HANDOFF.  AV for head-even drains
#    during its own phase (lag ~3 slots), head-odd's pts buffer in SBUF and
#    drain during the next phase, both through ONE PSUM ctx accumulator.
#    At chain end one ACT copy moves [128,1024] (ctx rows + ones-replicated
#    denominator) PSUM->SBUF; the reciprocal/shift/multiply tail runs a few
#    slots later as before.
#
# PSUM: s-ring 3 x [128,1024] (6 banks, shared with projections) + 1 ctx
# (2 banks) = 8 banks exactly.
from contextlib import ExitStack

import numpy as np
import ml_dtypes

import concourse.bass as bass
import concourse.mybir as mybir
import concourse.tile as tile
from concourse import bacc
from concourse.bass_utils import run_bass_kernel_spmd

BF16 = mybir.dt.bfloat16
F32 = mybir.dt.float32
Exp = mybir.ActivationFunctionType.Exp
MULT = mybir.AluOpType.mult
ADD = mybir.AluOpType.add

B, N, D, H = 4, 2048, 512, 8
DK = D // H          # 64
NQ = N // 2          # 1024 query rows per core
NKT = N // 128       # 16 k tiles
NP = H // 2          # 4 head pairs

# exp bit-trick constants: low16(fp32(x*K8 + BIG)) == bf16 bits of exp(x/8)
K8 = float(np.float32(128.0 / np.log(2.0) / 8.0))          # 23.0827
BIG = float(np.float32(1.5 * 2**23 + (127 << 7) - 8))      # fixed-point + bias


def build_nc():
    nc = bacc.Bacc("TRN2", target_bir_lowering=False)

    qT = nc.dram_tensor("qT", (D, NQ), BF16, kind="ExternalInput")
    kT = nc.dram_tensor("kT", (D, N), BF16, kind="ExternalInput")
    vT = nc.dram_tensor("vT", (D, N), BF16, kind="ExternalInput")
    wq = nc.dram_tensor("wq", (D, D), BF16, kind="ExternalInput")
    wk = nc.dram_tensor("wk", (D, D), BF16, kind="ExternalInput")
    wv = nc.dram_tensor("wv", (D, D), BF16, kind="ExternalInput")
    wo = nc.dram_tensor("wo", (D, D), BF16, kind="ExternalInput")
    bq = nc.dram_tensor("bq", (D, 1), F32, kind="ExternalInput")
    bk = nc.dram_tensor("bk", (D, 1), F32, kind="ExternalInput")
    bv = nc.dram_tensor("bv", (1, D), BF16, kind="ExternalInput")
    bo = nc.dram_tensor("bo", (1, D), BF16, kind="ExternalInput")
    out = nc.dram_tensor("out", (NQ, D), BF16, kind="ExternalOutput")

    with tile.TileContext(nc) as tc:
        with ExitStack() as ctx:
            emit(ctx, tc, qT, kT, vT, wq, wk, wv, wo, bq, bk, bv, bo, out)
    nc.compile()
    return nc


def emit(ctx, tc, qT, kT, vT, wq, wk, wv, wo, bq, bk, bv, bo, out):
    nc = tc.nc
    consts = ctx.enter_context(tc.tile_pool(name="consts", bufs=1))
    # odd-head pts (ACT exp, bf16) live up to a full phase before their AV
    p_pool = ctx.enter_context(tc.tile_pool(name="p_pool", bufs=17))
    # even-head DVE-trick tiles (f32, bitcast-read); consumed within ~3 slots
    y_pool = ctx.enter_context(tc.tile_pool(name="y_pool", bufs=4))
    post = ctx.enter_context(tc.tile_pool(name="post", bufs=1))
    outs = ctx.enter_context(tc.tile_pool(name="outs", bufs=4))
    # PSUM: shared 3-deep s-ring (S pairs + projections, 6 banks) + 1 ctx
    # (2 banks) = 8 banks.  (A per-parity 1-deep split was tried and lost
    # ~19us: exp latency ~1.2us needs >=1.5 slots of ring slack.)
    s_pool = ctx.enter_context(tc.tile_pool(name="s_pool", bufs=3, space="PSUM"))
    c_pool = ctx.enter_context(tc.tile_pool(name="c_pool", bufs=1, space="PSUM"))
    dram = ctx.enter_context(tc.tile_pool(name="dram", bufs=1, space="DRAM"))

    # ---- inputs (DMA order = first-use order; big tensors in halves) -------
    def load(name, shape, dt_, src_ap, eng=None):
        t = consts.tile(shape, dt_, name=name)
        (eng or nc.sync).dma_start(out=t, in_=src_ap)
        return t

    def load_halves(name, shape, dt_, dram_t, n, parts=2):
        t = consts.tile(shape, dt_, name=name)
        h = n // parts
        for i in range(parts):
            nc.sync.dma_start(
                out=t[:, :, i * h:(i + 1) * h],
                in_=dram_t[:, i * h:(i + 1) * h].rearrange(
                    "(c p) n -> p c n", p=128))
        return t

    def load_part(t, dram_t, n0, n1):
        nc.sync.dma_start(
            out=t[:, :, n0:n1],
            in_=dram_t[:, n0:n1].rearrange("(c p) n -> p c n", p=128))

    # DMA order tracks the fine-grained startup chain: the first S matmul
    # needs only [wk, kT cols 0:512, bk, wq, qT cols 0:512, bq] — 2.5MB —
    # so the remaining halves queue behind those.
    wk_s = load("wk_s", [128, 4, D], BF16, wk[:].rearrange("(c p) d -> p c d", p=128))
    kT_s = consts.tile([128, 4, N], BF16, name="kT_s")
    load_part(kT_s, kT, 0, 512)
    bk_s = load("bk_s", [128, 4, 1], F32, bk[:].rearrange("(c p) o -> p c o", p=128))
    wq_s = load("wq_s", [128, 4, D], BF16, wq[:].rearrange("(c p) d -> p c d", p=128))
    qT_s = consts.tile([128, 4, NQ], BF16, name="qT_s")
    load_part(qT_s, qT, 0, 512)
    bq_s = load("bq_s", [128, 4, 1], F32, bq[:].rearrange("(c p) o -> p c o", p=128))
    load_part(kT_s, kT, 512, 1024)
    load_part(qT_s, qT, 512, 1024)
    wv_s = load("wv_s", [128, 4, D], BF16, wv[:].rearrange("(c p) d -> p c d", p=128))
    bv_bc = load("bv_bc", [128, D], BF16, bv[:].to_broadcast((128, D)))
    load_part(kT_s, kT, 1024, 1536)
    load_part(kT_s, kT, 1536, 2048)
    vT_s = load_halves("vT_s", [128, 4, N], BF16, vT, N, parts=4)
    wo_s = load("wo_s", [128, 4, D], BF16, wo[:].rearrange("(c p) d -> p c d", p=128))
    bo_s = load("bo_s", [1, D], BF16, bo[:])

    ones1 = consts.tile([1, 128], BF16)
    nc.vector.memset(ones1, 1.0)
    ztile = consts.tile([128, 512], BF16)
    nc.vector.memset(ztile, 0.0)

    # tiny dummy exp: pulls the ~2.7us ACT_TABLE_LOAD into the DMA window
    tl = consts.tile([128, 16], F32)
    nc.scalar.activation(tl, ztile[:, 0:16], Exp, scale=1.0)

    KT_s = consts.tile([128, 4, N], BF16)     # K^T, d on partitions
    QT_s = consts.tile([128, 4, NQ], BF16)    # Q^T, d on partitions
    # V with k on partitions; per (kt, head) a 128-wide stationary block:
    # even heads [V(64) | ones(64)], odd heads [ones|V].  The ones half
    # replicates the softmax denominator onto the 64 partitions opposite
    # the ctx rows, so normalization needs no partition broadcast.
    V_s = consts.tile([128, NKT, H, 128], BF16)
    ctxn_s = consts.tile([128, 4, NQ], BF16)  # normalized ctx^T

    V_pairs = V_s[:].rearrange("p t (j par) w -> p t par j w", par=2)
    nc.vector.memset(V_pairs[:, :, 0, :, 64:128], 1.0)  # even heads: ones right
    nc.vector.memset(V_pairs[:, :, 1, :, 0:64], 1.0)    # odd heads: ones left

    # ---- projections --------------------------------------------------------
    def emit_kproj_half(dt, kh):  # one kT half: 8 MMs, one st ring slot
        st = s_pool.tile([128, 1024], F32, tag="s", name="st_k")
        for kc in range(2):
            for cc in range(4):
                nc.tensor.matmul(
                    st[:, kc * 512:(kc + 1) * 512],
                    lhsT=wk_s[:, cc, dt * 128:(dt + 1) * 128],
                    rhs=kT_s[:, cc, kh * 1024 + kc * 512:
                             kh * 1024 + (kc + 1) * 512],
                    start=(cc == 0), stop=(cc == 3))
        nc.scalar.add(
            KT_s[:, dt, kh * 1024:(kh + 1) * 1024], st, bk_s[:, dt, :])

    def emit_qproj(dt):  # 8 MMs, one st ring slot
        st = s_pool.tile([128, 1024], F32, tag="s", name="st_q")
        for qc in range(2):
            for cc in range(4):
                nc.tensor.matmul(
                    st[:, qc * 512:(qc + 1) * 512],
                    lhsT=wq_s[:, cc, dt * 128:(dt + 1) * 128],
                    rhs=qT_s[:, cc, qc * 512:(qc + 1) * 512],
                    start=(cc == 0), stop=(cc == 3))
        nc.scalar.add(QT_s[:, dt, :], st, bq_s[:, dt, :])

    def emit_vproj_pair(g):  # V projection for k tiles 2g, 2g+1
        st = s_pool.tile([128, 1024], F32, tag="s", name="st_v")
        for sub in range(2):
            kt = 2 * g + sub
            sl = st[:, sub * 512:(sub + 1) * 512]
            for cc in range(4):
                nc.tensor.matmul(
                    sl,
                    lhsT=vT_s[:, cc, kt * 128:(kt + 1) * 128],
                    rhs=wv_s[:, cc, :],
                    start=(cc == 0), stop=(cc == 3))
            sl_pairs = sl.rearrange("p (j par w) -> p par j w", par=2, w=64)
            bv_pairs = bv_bc[:].rearrange("p (j par w) -> p par j w",
                                          par=2, w=64)
            vt_pairs = V_s[:, kt].rearrange("p (j par) w -> p par j w", par=2)
            nc.vector.tensor_add(
                vt_pairs[:, 0, :, 0:64], sl_pairs[:, 0], bv_pairs[:, 0])
            nc.vector.tensor_add(
                vt_pairs[:, 1, :, 64:128], sl_pairs[:, 1], bv_pairs[:, 1])

    # ---- attention ----------------------------------------------------------
    def emit_s_pair(p, kt, dve_even):
        """Row-tiled S^T pair for heads (2p, 2p+1) at k-tile kt, plus exp.

        The 4 matmuls alternate partition halves (e,qc0),(o,qc0),(e,qc1),
        (o,qc1): consecutive MMs land on disjoint 64-row PE tiles (T0/T8)
        and run concurrently.  Returns (rhs_e, rhs_o): per-head AV rhs APs.
        """
        st_e = s_pool.tile([128, 1024], F32, tag="s", name="st_e")
        st_o = s_pool.tile([128, 1024], F32, tag="s", name="st_o")
        kcol = slice(kt * 128, (kt + 1) * 128)
        for qc in range(2):
            qs = slice(qc * 512, (qc + 1) * 512)
            nc.tensor.matmul(
                st_e[:, qs], lhsT=KT_s[0:64, p, kcol], rhs=QT_s[0:64, p, qs],
                start=True, stop=True)
            nc.tensor.matmul(
                st_o[:, qs], lhsT=KT_s[64:128, p, kcol], rhs=QT_s[64:128, p, qs],
                start=True, stop=True)
        if dve_even:
            y = y_pool.tile([128, 1024], F32, tag="y", name="y_e")
            nc.vector.tensor_scalar(y, st_e, K8, BIG, op0=MULT, op1=ADD)
            rhs_e = y[:].bitcast(BF16).rearrange(
                "p (n two) -> p n two", two=2)[:, :, 0]
        else:
            pt_e = p_pool.tile([128, 1024], BF16, tag="pe", bufs=2, name="pt_e")
            nc.scalar.activation(pt_e, st_e, Exp, scale=0.125)
            rhs_e = pt_e[:]
        pt_o = p_pool.tile([128, 1024], BF16, tag="p", name="pt_o")
        nc.scalar.activation(pt_o, st_o, Exp, scale=0.125)
        return rhs_e, pt_o[:]

    def emit_av(h, kt, rhs, ctx_ps):
        for qc in range(2):
            nc.tensor.matmul(
                ctx_ps[:, qc * 512:(qc + 1) * 512],
                lhsT=V_s[:, kt, h, :],
                rhs=rhs[:, qc * 512:(qc + 1) * 512],
                start=(kt == 0), stop=(kt == NKT - 1))

    # ---- per-head normalize -------------------------------------------------
    # At chain end one ACT copy stages [128,1024] (ctx half + denominator
    # half) PSUM->SBUF, freeing the single ctx accumulator.  The deferred
    # tail does reciprocal (partitions 0-63 only; partition-shift via
    # SBUF->SBUF DMA on whichever side needs it) and the multiply into ctxn.
    norm7 = {}

    def emit_handoff(h, ctx_ps):
        cc = post.tile([128, NQ], F32, tag="cc", bufs=2, name="cc")
        nc.scalar.copy(out=cc, in_=ctx_ps)
        even = (h % 2 == 0)
        cl, dl = (0, 64) if even else (64, 0)   # ctx / denom partition bases
        if h == H - 1:
            # last head: fold normalize into the output projection — stage
            # raw ctx bf16 (matmul lhsT) + reciprocals transposed onto
            # partitions via a 4KB DRAM bounce.
            def tail():
                rc = post.tile([128, NQ], F32, tag="rc", name="rc")
                nc.vector.reciprocal_approx_fast(out=rc[0:64, :],
                                                 in_=cc[0:64, :])
                ctxc = post.tile([128, NQ], BF16, tag="ctxc7", name="ctxc7")
                nc.vector.tensor_copy(out=ctxc[64:128, :], in_=cc[64:128, :])
                norm7["ctxc"] = ctxc
                dr = dram.tile([1, NQ], F32, tag="dr", name="dr")
                nc.sync.dma_start(out=dr, in_=rc[0:1, :])
                rcol = post.tile([128, 8], F32, tag="rcol", name="rcol")
                nc.sync.dma_start(
                    out=rcol,
                    in_=dr[:].rearrange("o (f p) -> (o p) f", p=128))
                norm7["rcol"] = rcol
            return tail

        def tail():
            dt = h // 2
            d = cc
            if dl != 0:
                den2 = post.tile([128, NQ], F32, tag="shift", name="den2")
                nc.sync.dma_start(out=den2[0:64, :], in_=cc[dl:dl + 64, :])
                d = den2
            rc = post.tile([128, NQ], F32, tag="rc", name="rc")
            nc.vector.reciprocal_approx_fast(out=rc[0:64, :], in_=d[0:64, :])
            if cl != 0:
                rc2 = post.tile([128, NQ], F32, tag="shift", name="rc2")
                nc.sync.dma_start(out=rc2[cl:cl + 64, :], in_=rc[0:64, :])
                rc = rc2
            nc.vector.tensor_mul(ctxn_s[cl:cl + 64, dt, :],
                                 cc[cl:cl + 64, :], rc[cl:cl + 64, :])
        return tail

    # ---- output projection (unchanged from baseline) ------------------------
    def emit_outproj_main(g, pool, tag):
        st = pool.tile([128, 1024], F32, tag=tag, name="st_o")
        for sub in range(2):
            nt = g * 2 + sub
            sl = st[:, sub * 512:(sub + 1) * 512]
            for dc in range(3):
                nc.tensor.matmul(
                    sl,
                    lhsT=ctxn_s[:, dc, nt * 128:(nt + 1) * 128],
                    rhs=wo_s[:, dc, :],
                    start=(dc == 0), stop=False)
            nc.tensor.matmul(
                sl,
                lhsT=ctxn_s[0:64, 3, nt * 128:(nt + 1) * 128],
                rhs=wo_s[0:64, 3, :],
                start=False, stop=False)
            nc.tensor.matmul(sl, lhsT=ones1, rhs=bo_s, start=False, stop=True)
        return st

    def emit_fin_pre(g, st):
        # stage the main sums to SBUF promptly (ACT; frees nothing but gets
        # the copies off the post-fold critical path)
        ots = []
        for sub in range(2):
            sl = st[:, sub * 512:(sub + 1) * 512]
            ot = outs.tile([128, D], BF16, tag="o", bufs=6, name="ot")
            nc.scalar.copy(out=ot, in_=sl)
            ots.append(ot)
        return ots

    def emit_fin_mm(g, st, sub):
        nt = g * 2 + sub
        nc.tensor.matmul(
            st[:, sub * 512:(sub + 1) * 512],
            lhsT=norm7["ctxc"][64:128, nt * 128:(nt + 1) * 128],
            rhs=wo_s[64:128, 3, :],
            start=True, stop=True)

    def emit_fin_merge(g, st, ots, sub):
        # merge = (h7_partial * recip7[n]) + main: the scale rides the ACT
        # copy (per-partition scale AP), the add is a cheap bf16 DVE op —
        # splitting engines keeps the 8 merges off a single serial DVE queue
        nt = g * 2 + sub
        sl = st[:, sub * 512:(sub + 1) * 512]
        tm = outs.tile([128, D], BF16, tag="tm", bufs=3, name="tm")
        nc.scalar.activation(tm, sl, mybir.ActivationFunctionType.Copy,
                             scale=norm7["rcol"][:, nt:nt + 1])
        ot2 = outs.tile([128, D], BF16, tag="o2", bufs=3, name="ot2")
        nc.vector.tensor_add(ot2, tm, ots[sub])
        nc.sync.dma_start(out=out[nt * 128:(nt + 1) * 128, :], in_=ot2)

    # ---- schedule -----------------------------------------------------------
    # warm-up: dummy matmuls during the initial DMA window keep the PE's HAM
    # clock gate at 8/8 until the first projection matmuls are DMA-ready
    wst = s_pool.tile([128, 1024], F32, tag="s", name="wst")
    for i in range(12):
        nc.tensor.matmul(wst[:, 0:512], lhsT=ztile[:, 0:128], rhs=ztile,
                         start=(i == 0), stop=(i == 11))
    # startup: interleave the first K/Q projection halves with per-half
    # biases on alternating engines — S(0,0) needs only kc0+qc0+qc1, so the
    # serial [proj -> bias -> proj -> bias] ACT chain is cut down
    stk = s_pool.tile([128, 1024], F32, tag="s", name="stk")
    stq = s_pool.tile([128, 1024], F32, tag="s", name="stq")
    for half in range(2):
        for cc in range(4):
            nc.tensor.matmul(
                stk[:, half * 512:(half + 1) * 512],
                lhsT=wk_s[:, cc, 0:128],
                rhs=kT_s[:, cc, half * 512:(half + 1) * 512],
                start=(cc == 0), stop=(cc == 3))
        nc.vector.tensor_scalar_add(
            KT_s[:, 0, half * 512:(half + 1) * 512],
            stk[:, half * 512:(half + 1) * 512], bk_s[:, 0, :])
        for cc in range(4):
            nc.tensor.matmul(
                stq[:, half * 512:(half + 1) * 512],
                lhsT=wq_s[:, cc, 0:128],
                rhs=qT_s[:, cc, half * 512:(half + 1) * 512],
                start=(cc == 0), stop=(cc == 3))
        nc.scalar.add(QT_s[:, 0, half * 512:(half + 1) * 512],
                      stq[:, half * 512:(half + 1) * 512], bq_s[:, 0, :])

    # Projection prefetch placement per phase (16 pair-kt slots each).
    # DMA arrival order gates phase-0 placements (kT half 2, all of vT
    # arrive while phase 0 runs).
    def phase_mid(p):
        # K^T/Q^T for phase p+1 must complete during phase p.
        # kproj(p, 1) covers k-tiles 8-15 of phase p, so it can run INSIDE
        # phase p (by slot ~7) — sheds load from the crowded previous phase.
        if p == 0:
            mid = {1: lambda: emit_kproj_half(0, 1),
                   12: lambda: emit_kproj_half(1, 0),
                   14: lambda: emit_qproj(1)}
            for g in range(7):
                mid[3 + g] = (lambda g=g: emit_vproj_pair(g))
            mid[11] = lambda: emit_vproj_pair(7)
            return mid
        if p == 1:
            return {2: lambda: emit_kproj_half(1, 1),
                    6: lambda: emit_kproj_half(2, 0),
                    10: lambda: emit_qproj(2)}
        if p == 2:
            return {2: lambda: emit_kproj_half(2, 1),
                    6: lambda: emit_kproj_half(3, 0),
                    10: lambda: emit_qproj(3)}
        return {3: lambda: emit_kproj_half(3, 1)}

    # AV consumption: strictly chain-sequential (head 0 fully, then 1, ...)
    # through the single ctx accumulator.  prod[h][kt] = (rhs, produced_slot);
    # an entry is poppable LAG slots after production (so the PE never
    # head-blocks waiting on its exp).  Hand-off at kt==15, tail deferred to
    # the next chain's kt==4.
    LAG = 2
    prod = {h: {} for h in range(H)}
    cons = {"h": 0, "kt": 0, "n": 0, "ctx": None}
    tails = {}
    n_prod = [0]

    def next_entry():
        if cons["h"] >= H:
            return None
        return prod[cons["h"]].get(cons["kt"])

    def pop_av(now_slot):
        e = next_entry()
        if e is None or (now_slot is not None and now_slot < e[1] + LAG):
            return False
        h, kt = cons["h"], cons["kt"]
        if kt == 0:
            # last chain: take a (by-then free) s-ring slot so its first AV
            # doesn't serialize behind the previous chain's hand-off copy
            pool, tag = (s_pool, "s") if h == H - 1 else (c_pool, "c")
            cons["ctx"] = pool.tile([128, 1024], F32, tag=tag, name="ctx_ps")
        emit_av(h, kt, e[0], cons["ctx"])
        cons["n"] += 1
        if kt == 4 and h >= 1:
            t = tails.pop(h - 1, None)
            if t:
                t()
        if kt == NKT - 1:
            tails[h] = emit_handoff(h, cons["ctx"])
            cons["h"] += 1
            cons["kt"] = 0
        else:
            cons["kt"] += 1
        return True

    for p in range(NP):
        mid = phase_mid(p)
        for kt in range(NKT):
            slot = p * NKT + kt
            if kt in mid:
                mid[kt]()
            dve_even = not (p == 0 and kt in (0, 2, 5, 7, 10, 12, 15))
            rhs_e, rhs_o = emit_s_pair(p, kt, dve_even)
            prod[2 * p][kt] = (rhs_e, slot)
            prod[2 * p + 1][kt] = (rhs_o, slot)
            n_prod[0] += 2
            target = max(6, 11 - kt)
            while n_prod[0] - cons["n"] > target and pop_av(slot):
                pass
    # Final drain: chain 6 remainder + chain 7, with outproj mains g0-g2
    # interleaved into the AV stream (tail(6) fires at chain-7 kt4,
    # unblocking the h6 half-column they read).  g2 rides the c-ring slot
    # freed by chain 6's hand-off; g3's s-slot frees at chain 7's hand-off.
    main_cfg = [("s", s_pool), ("s", s_pool), ("c", c_pool), ("s", s_pool)]
    sts, fots = [], []

    def emit_main(g):
        tag, pool = main_cfg[g]
        st = emit_outproj_main(g, pool, tag)
        sts.append(st)
        fots.append(emit_fin_pre(g, st))

    while pop_av(None):
        if (cons["h"] == 7 and len(sts) < 3
                and cons["kt"] >= 6 + 3 * len(sts)):
            emit_main(len(sts))
    t = tails.pop(6, None)
    if t:
        t()
    tails.pop(7)()
    while len(sts) < 4:
        emit_main(len(sts))
    # round-robin the h7-partial matmuls and merges across the four group
    # tiles so no fin matmul waits on a same-tile merge read
    for sub in range(2):
        for g in range(4):
            emit_fin_mm(g, sts[g], sub)
        for g in range(4):
            emit_fin_merge(g, sts[g], fots[g], sub)


_NC_CACHE = None


def _get_nc():
    global _NC_CACHE
    if _NC_CACHE is None:
        _NC_CACHE = build_nc()
    return _NC_CACHE


def make_in_maps(query, key, value, Wq, bq, Wk, bk, Wv, bv, Wo, bo):
    bf = ml_dtypes.bfloat16
    f = np.float32
    query = np.asarray(query, f)
    key = np.asarray(key, f)
    value = np.asarray(value, f)
    shared = {
        "wq": np.asarray(Wq, f).astype(bf),
        "wk": np.asarray(Wk, f).astype(bf),
        "wv": np.asarray(Wv, f).astype(bf),
        "wo": np.asarray(Wo, f).astype(bf),
        "bq": np.asarray(bq, f).reshape(D, 1),
        "bk": np.asarray(bk, f).reshape(D, 1),
        "bv": np.asarray(bv, f).astype(bf).reshape(1, D),
        "bo": np.asarray(bo, f).astype(bf).reshape(1, D),
    }
    kTs = [np.ascontiguousarray(key[b].T).astype(bf) for b in range(B)]
    vTs = [np.ascontiguousarray(value[b].T).astype(bf) for b in range(B)]
    in_maps = []
    for c in range(8):
        b, half = c // 2, c % 2
        m = dict(shared)
        m["qT"] = np.ascontiguousarray(
            query[b, half * NQ:(half + 1) * NQ, :].T).astype(bf)
        m["kT"] = kTs[b]
        m["vT"] = vTs[b]
        in_maps.append(m)
    return in_maps


def run(inputs, trace=False):
    nc = _get_nc()
    in_maps = make_in_maps(**inputs)
    res = run_bass_kernel_spmd(nc, in_maps, core_ids=list(range(8)), trace=trace)
    out = np.empty((B, N, D), np.float32)
    for c in range(8):
        b, half = c // 2, c % 2
        out[b, half * NQ:(half + 1) * NQ, :] = np.asarray(
            res.results[c]["out"], dtype=np.float32)
    return out, res


def kernel(**inputs):
    out, _ = run(inputs, trace=False)
    return out
